# Initial kernel scaffold
#
"""Trainium2 Bass kernel for nn_EnhancedTextAttentionBlock.

Self-contained: takes FULL inputs (as in reference.setup_inputs()), shards
across 8 NeuronCores internally, returns the FULL [2, 256, 48, 48] output.

Sharding: core c handles batch b = c // 4 and query-token block k = c % 4
(576 of the 2304 spatial tokens). K/V (and their layernorm) are computed for
the full token set on every core; the query-side path uses host-sliced
inputs, so a single SPMD program serves all 8 cores with no collectives.

Key algebraic restructurings (exact, not approximations):
- The positional encoding pe depends only on (c, w), so the 3x3 conv output
  has only 3 distinct rows (top / middle / bottom). The conv collapses to
  three 1-D convs along w with kh-summed kernels == matmuls over an im2col
  of a [C, 48] tensor.
- v-projection bias commutes through the softmax-normalized attention:
  attn @ (v + 1 v_b^T) = attn @ v + 1 v_b^T, so v_b folds into an effective
  output bias o_b_eff = o_b + v_b @ o_w.T on the host.
- Softmax denominators ride along as an extra ones-column of v and an extra
  l-transport column of the output projection, landing the per-token 1/l in
  token-major layout where it is a cheap per-partition rescale.
- Softmax max-subtraction is skipped: LN'd activations through 0.02-scale
  weights give |scores| < ~2, where exp() is exactly safe in fp32.
"""
import math
import numpy as np

import concourse.bass as bass
import concourse.tile as tile
from concourse import bacc, mybir
from concourse.bass_utils import run_bass_kernel_spmd

import os as _os
F32 = mybir.dt.float32
_PREC = _os.environ.get("KERNEL_PREC", "tuned")
if _os.environ.get("KERNEL_F32") == "1":
    _PREC = "f32"
# per-stage matmul dtypes: projections, scores, attention-value, out-proj
_R = mybir.dt.float32r
PROJ_DT = F32 if _PREC in ("f32", "pf32", "pof32", "tuned") else _R
VPROJ_DT = PROJ_DT  # v matmul shares kn operand; dtype must match
SCORES_DT = F32 if _PREC in ("f32", "sf32") else _R
AV_DT = F32 if _PREC in ("f32", "af32") else _R
OPROJ_DT = F32 if _PREC in ("f32", "af32", "of32", "pof32", "tuned") else _R
F32R = _R
AF = mybir.ActivationFunctionType
OP = mybir.AluOpType

B, C, H, W, T = 2, 256, 48, 48, 512
NH, HD = 8, 32
S = H * W              # 2304 tokens
NQ = S // 4            # 576 q tokens per core
SCALE = HD ** -0.5
IT = 288               # i-tile (two per q block)
MC = 96                # epilogue chunk
EPS = 1e-5

# cvecs column indices (c-major [256, 1] vectors packed into one input)
CV_TMB1, CV_L1G, CV_L1B, CV_TMB2, CV_L2GN, CV_L2BN, \
    CV_NQG, CV_NQB, CV_NKVG, CV_NKVB, CV_CONVB, CV_GW = range(12)
# rowvecs (token-major prebroadcast [128, 256] rows)
RV_OB, RV_NOG, RV_NOB = range(3)


def _partition_stats(nc, pools, x_tiles, n_free, nch, eps_sb, ones_sb, tag):
    """Mean/rsqrt(var) across the partition+chunk (256-channel) dim of
    c-major tiles. x_tiles[cc] = AP [128, n_free]. Returns (mu_bc, rs_bc)
    [128, n_free] broadcast tiles."""
    sb, ps = pools
    HALF = 512
    nhalf = (n_free + HALF - 1) // HALF
    mu = sb.tile([1, n_free], F32, tag="st_mu")
    var = sb.tile([1, n_free], F32, tag="st_var")
    for hf in range(nhalf):
        f0 = hf * HALF
        fn = min(HALF, n_free - f0)
        sum_ps = ps.tile([1, HALF], F32, tag="stsum")
        sumsq_ps = ps.tile([1, HALF], F32, tag="stsumsq")
        for cc in range(nch):
            sq = sb.tile([128, HALF], F32, tag="scrA")
            nc.vector.tensor_mul(sq[:, :fn], x_tiles[cc][:, f0:f0 + fn],
                                 x_tiles[cc][:, f0:f0 + fn])
            nc.tensor.matmul(sum_ps[:, :fn], ones_sb[:], x_tiles[cc][:, f0:f0 + fn],
                             start=(cc == 0), stop=(cc == nch - 1))
            nc.tensor.matmul(sumsq_ps[:, :fn], ones_sb[:], sq[:, :fn],
                             start=(cc == 0), stop=(cc == nch - 1))
        nc.vector.tensor_scalar_mul(mu[:, f0:f0 + fn], sum_ps[:, :fn], 1.0 / 256.0)
        nc.vector.tensor_scalar_mul(var[:, f0:f0 + fn], sumsq_ps[:, :fn], 1.0 / 256.0)
    scr = sb.tile([1, n_free], F32, tag="st_scr")
    nc.vector.tensor_mul(scr[:], mu[:], mu[:])
    nc.vector.tensor_sub(var[:], var[:], scr[:])
    scr2 = sb.tile([1, n_free], F32, tag="st_scr")
    nc.scalar.activation(scr2[:], var[:], AF.Ln, bias=eps_sb[:], scale=1.0)
    rs = sb.tile([1, n_free], F32, tag="st_var2")
    nc.scalar.activation(rs[:], scr2[:], AF.Exp, scale=-0.5)
    mu_bc = sb.tile([128, n_free], F32, tag="st_mubc")
    nc.gpsimd.partition_broadcast(mu_bc[:], mu[:])
    rs_bc = sb.tile([128, n_free], F32, tag="st_rsbc")
    nc.gpsimd.partition_broadcast(rs_bc[:], rs[:])
    return mu_bc, rs_bc


def build_bass():
    nc = bacc.Bacc("TRN2", target_bir_lowering=False, debug=False,
                   enable_asserts=True, num_devices=8)
    di = {}

    def inp(name, shape, dt=F32):
        di[name] = nc.dram_tensor(name, shape, dt, kind="ExternalInput")
        return di[name]

    xk = inp("xk", [C, S])
    xq = inp("xq", [C, NQ])
    xqres = inp("xqres", [NQ, C])
    text = inp("text", [T, 1])
    tmw1 = inp("tmw1", [T, C])
    tmw2 = inp("tmw2", [C, C])
    cvecs = inp("cvecs", [C, 12])
    dvecs = inp("dvecs", [C, 2])
    pe = inp("pe", [C, W])
    w3 = inp("w3", [3, 768, C])
    qwT = inp("qwT", [C, C], PROJ_DT)
    kwT = inp("kwT", [C, C], PROJ_DT)
    vwT = inp("vwT", [C, C], VPROJ_DT)
    owx = inp("owx", [128, NH, 258], OPROJ_DT)
    rowvecs = inp("rowvecs", [128, 3, C])
    selmask = inp("selmask", [128, 2, NQ])
    y = nc.dram_tensor("y", [NQ, C], F32, kind="ExternalOutput")

    with tile.TileContext(nc) as tc:
        _build_tile(nc, tc, di, y)
    nc.compile()
    return nc


def _build_tile(nc, tc, di, y):
    with tc.tile_pool(name="cons", bufs=1) as cons, \
         tc.tile_pool(name="dram", bufs=1, space="DRAM") as dram:
        # ---- persistent small tiles ----
        ones_sb = cons.tile([128, 1], F32)
        nc.vector.memset(ones_sb[:], 1.0)
        eps1 = cons.tile([1, 1], F32)
        nc.vector.memset(eps1[:], EPS)
        cv = cons.tile([128, 2, 12], F32)
        nc.sync.dma_start(out=cv, in_=di["cvecs"].rearrange("(c p) v -> p c v", p=128))
        dv = cons.tile([128, 2, 2], F32)
        nc.sync.dma_start(out=dv, in_=di["dvecs"].rearrange("(c p) v -> p c v", p=128))
        pe_sb = cons.tile([128, 2, W], F32)
        nc.sync.dma_start(out=pe_sb, in_=di["pe"].rearrange("(c p) w -> p c w", p=128))
        qw_sb = cons.tile([128, 2, C], PROJ_DT)
        kw_sb = cons.tile([128, 2, C], PROJ_DT)
        vw_sb = cons.tile([128, 2, C], VPROJ_DT)
        ow_sb = cons.tile([128, NH, 258], OPROJ_DT)
        rv_sb = cons.tile([128, 3, C], F32)
        sel_sb = cons.tile([128, 2, NQ], F32)
        nc.sync.dma_start(out=sel_sb, in_=di["selmask"][:, :, :])
        posrow = cons.tile([128, 2, 3, W], F32)   # (cc, rowtype, w)
        dtop = cons.tile([128, 2, W], F32)
        dbot = cons.tile([128, 2, W], F32)
        qn_sb = cons.tile([128, 2, NQ], PROJ_DT)
        v_tok = cons.tile([128, 18, NH, 33], AV_DT)
        out_acc = cons.tile([MC, 6, C], F32)
        eg_sb = cons.tile([MC, 6], F32)          # exp(-gate logits)
        gate_sb = cons.tile([MC, 6], F32)
        xqres_sb = cons.tile([MC, 6, C], F32)
        ktd = dram.tile([4, 32, 2, S], SCORES_DT)     # per-pair kT in DRAM
        qtd = dram.tile([4, 32, 2, NQ], SCORES_DT)

        # ================= Phase A/B/C/D: prologue ==================
        with tc.tile_pool(name="ph", bufs=1) as ph, \
             tc.tile_pool(name="pps", bufs=2, space="PSUM") as pps, \
             tc.tile_pool(name="spps", bufs=1, space="PSUM") as spps:
            # ---- text modulation MLP (c-major) ----
            text_sb = ph.tile([128, 4, 1], F32)
            nc.sync.dma_start(out=text_sb,
                              in_=di["text"].rearrange("(k p) o -> p k o", p=128))
            w1_sb = ph.tile([128, 4, C], F32, tag="scrB")
            nc.sync.dma_start(out=w1_sb,
                              in_=di["tmw1"].rearrange("(k p) d -> p k d", p=128))
            w2_sb = ph.tile([128, 2, C], F32, tag="scrC")
            nc.sync.dma_start(out=w2_sb,
                              in_=di["tmw2"].rearrange("(k p) d -> p k d", p=128))

            def cmajor_mlp_layer(x_col, w_sb, nkc, bias_col, tag):
                # out[c2] = sum_k w_sb[k, c2] x_col[k]  -> [128, 2, 1] + bias
                h_col = ph.tile([128, 2, 1], F32, tag=f"{tag}_h")
                for c2c in range(2):
                    h_ps = pps.tile([128, 1], F32, tag="mlpps")
                    for kc in range(nkc):
                        nc.tensor.matmul(
                            h_ps[:, :], w_sb[:, kc, c2c * 128:(c2c + 1) * 128],
                            x_col[:, kc, :], start=(kc == 0), stop=(kc == nkc - 1))
                    nc.scalar.activation(h_col[:, c2c, :], h_ps[:, :], AF.Identity,
                                         bias=bias_col[:, c2c, :])
                return h_col

            def cmajor_ln_stats(h_col, tag):
                # 256-dim stats of [128, 2, 1] -> broadcast [128, 1] mu, rs
                sum_ps = spps.tile([1, 1], F32, tag="ssum")
                sq_ps = spps.tile([1, 1], F32, tag="ssq")
                hsq = ph.tile([128, 2, 1], F32, tag=f"{tag}_hsq")
                nc.vector.tensor_mul(hsq[:], h_col[:], h_col[:])
                for cc in range(2):
                    nc.tensor.matmul(sum_ps[:, :], ones_sb[:], h_col[:, cc, :],
                                     start=(cc == 0), stop=(cc == 1))
                    nc.tensor.matmul(sq_ps[:, :], ones_sb[:], hsq[:, cc, :],
                                     start=(cc == 0), stop=(cc == 1))
                mu1 = ph.tile([1, 1], F32, tag=f"{tag}_mu1")
                nc.vector.tensor_scalar_mul(mu1[:], sum_ps[:, :], 1.0 / 256.0)
                var1 = ph.tile([1, 1], F32, tag=f"{tag}_var1")
                nc.vector.tensor_scalar_mul(var1[:], sq_ps[:, :], 1.0 / 256.0)
                musq1 = ph.tile([1, 1], F32, tag=f"{tag}_musq1")
                nc.vector.tensor_mul(musq1[:], mu1[:], mu1[:])
                nc.vector.tensor_sub(var1[:], var1[:], musq1[:])
                nc.scalar.activation(var1[:], var1[:], AF.Ln, bias=eps1[:], scale=1.0)
                nc.scalar.activation(var1[:], var1[:], AF.Exp, scale=-0.5)
                mu_b = ph.tile([128, 1], F32, tag=f"{tag}_mub")
                nc.gpsimd.partition_broadcast(mu_b[:], mu1[:])
                rs_b = ph.tile([128, 1], F32, tag=f"{tag}_rsb")
                nc.gpsimd.partition_broadcast(rs_b[:], var1[:])
                return mu_b, rs_b

            h1 = cmajor_mlp_layer(text_sb, w1_sb, 4, cv[:, :, CV_TMB1:CV_TMB1 + 1], "l1")
            mu_b, rs_b = cmajor_ln_stats(h1, "l1")
            h1n = ph.tile([128, 2, 1], F32, tag="h1n")
            for cc in range(2):
                nc.vector.tensor_sub(h1n[:, cc, :], h1[:, cc, :], mu_b[:])
                nc.vector.tensor_mul(h1n[:, cc, :], h1n[:, cc, :], rs_b[:])
                nc.scalar.activation(h1n[:, cc, :], h1n[:, cc, :], AF.Relu,
                                     bias=cv[:, cc, CV_L1B:CV_L1B + 1], scale=cv[:, cc, CV_L1G:CV_L1G + 1])
            h2 = cmajor_mlp_layer(h1n, w2_sb, 2, cv[:, :, CV_TMB2:CV_TMB2 + 1], "l2")
            mu2_b, rs2_b = cmajor_ln_stats(h2, "l2")
            mod = ph.tile([128, 2, 1], F32, tag="mod")
            for cc in range(2):
                nc.vector.tensor_sub(mod[:, cc, :], h2[:, cc, :], mu2_b[:])
                nc.vector.tensor_mul(mod[:, cc, :], mod[:, cc, :], rs2_b[:])
                # exp(-(g*xn + b)) via pre-negated g, b
                nc.scalar.activation(mod[:, cc, :], mod[:, cc, :], AF.Exp,
                                     bias=cv[:, cc, CV_L2BN:CV_L2BN + 1], scale=cv[:, cc, CV_L2GN:CV_L2GN + 1])
                nc.vector.tensor_scalar(mod[:, cc, :], mod[:, cc, :], 1.0, None, OP.add)
                nc.vector.reciprocal(mod[:, cc, :], mod[:, cc, :])

            # ---- conditional positional rows: 3 distinct conv rows ----
            w3_sb = ph.tile([128, 3, 6, C], F32, tag="bigbuf2")
            nc.sync.dma_start(out=w3_sb,
                              in_=di["w3"].rearrange("t (j p) m -> p t j m", p=128))
            # deferred non-critical loads (behind the prologue-critical DMAs)
            nc.sync.dma_start(out=qw_sb, in_=di["qwT"].rearrange("(c p) d -> p c d", p=128))
            nc.sync.dma_start(out=kw_sb, in_=di["kwT"].rearrange("(c p) d -> p c d", p=128))
            nc.sync.dma_start(out=vw_sb, in_=di["vwT"].rearrange("(c p) d -> p c d", p=128))
            nc.sync.dma_start(out=ow_sb, in_=di["owx"][:, :, :])
            nc.sync.dma_start(out=rv_sb, in_=di["rowvecs"][:, :, :])
            nc.sync.dma_start(out=xqres_sb,
                              in_=di["xqres"].rearrange("(k p) c -> p k c", p=MC))
            inrow = ph.tile([128, 2, W], F32)
            for cc in range(2):
                nc.vector.tensor_scalar_mul(inrow[:, cc, :], pe_sb[:, cc, :],
                                            mod[:, cc, 0:1])
            im2 = ph.tile([128, 6, W], F32, tag="scrC")
            nc.vector.memset(im2[:], 0.0)
            for kw in range(3):
                for cc in range(2):
                    j = kw * 2 + cc
                    if kw == 0:
                        nc.vector.tensor_copy(im2[:, j, 1:W], inrow[:, cc, 0:W - 1])
                    elif kw == 1:
                        nc.vector.tensor_copy(im2[:, j, :], inrow[:, cc, :])
                    else:
                        nc.vector.tensor_copy(im2[:, j, 0:W - 1], inrow[:, cc, 1:W])
            cps = pps.tile([128, 3, 2, W], F32, tag="projps")
            for t in range(3):
                for oc in range(2):
                    for j in range(6):
                        nc.tensor.matmul(cps[:, t, oc, :],
                                         w3_sb[:, t, j, oc * 128:(oc + 1) * 128],
                                         im2[:, j, :],
                                         start=(j == 0), stop=(j == 5))
            for cc in range(2):
                nc.scalar.activation(posrow[:, cc, :, :], cps[:, :, cc, :], AF.Identity,
                                     bias=cv[:, cc, CV_CONVB:CV_CONVB + 1])
                nc.vector.tensor_sub(dtop[:, cc, :], posrow[:, cc, 0, :],
                                     posrow[:, cc, 1, :])
                nc.vector.tensor_sub(dbot[:, cc, :], posrow[:, cc, 2, :],
                                     posrow[:, cc, 1, :])

            # ---- tokens (c-major) ----
            xk_sb = ph.tile([128, 2, S], F32, tag="bigbuf1")
            nc.sync.dma_start(out=xk_sb,
                              in_=di["xk"].rearrange("(c p) s -> p c s", p=128))
            tok = ph.tile([128, 2, S], F32)
            for cc in range(2):
                nc.vector.tensor_add(tok[:, cc, 0:W], xk_sb[:, cc, 0:W],
                                     posrow[:, cc, 0, :])
                mid = posrow[:, cc, 1:2, :].to_broadcast([128, H - 2, W])
                nc.vector.tensor_tensor(
                    tok[:, cc, W:S - W].rearrange("p (h w) -> p h w", w=W),
                    xk_sb[:, cc, W:S - W].rearrange("p (h w) -> p h w", w=W),
                    mid, OP.add)
                nc.vector.tensor_add(tok[:, cc, S - W:S], xk_sb[:, cc, S - W:S],
                                     posrow[:, cc, 2, :])
            xq_sb = ph.tile([128, 2, NQ], F32, tag="scrB")
            nc.sync.dma_start(out=xq_sb,
                              in_=di["xq"].rearrange("(c p) s -> p c s", p=128))
            tokq = ph.tile([128, 2, NQ], F32)
            seltmp = ph.tile([128, NQ], F32, tag="scrA")
            for cc in range(2):
                mid = posrow[:, cc, 1:2, :].to_broadcast([128, NQ // W, W])
                nc.vector.tensor_tensor(
                    tokq[:, cc, :].rearrange("p (h w) -> p h w", w=W),
                    xq_sb[:, cc, :].rearrange("p (h w) -> p h w", w=W),
                    mid, OP.add)
                nc.vector.tensor_tensor(
                    seltmp[:].rearrange("p (h w) -> p h w", w=W),
                    sel_sb[:, 0, :].rearrange("p (h w) -> p h w", w=W),
                    dtop[:, cc, None, :].to_broadcast([128, NQ // W, W]), OP.mult)
                nc.vector.tensor_add(tokq[:, cc, :], tokq[:, cc, :], seltmp[:])
                nc.vector.tensor_tensor(
                    seltmp[:].rearrange("p (h w) -> p h w", w=W),
                    sel_sb[:, 1, :].rearrange("p (h w) -> p h w", w=W),
                    dbot[:, cc, None, :].to_broadcast([128, NQ // W, W]), OP.mult)
                nc.vector.tensor_add(tokq[:, cc, :], tokq[:, cc, :], seltmp[:])

            # ---- layernorms (partition-dim stats) ----
            mu_bc, rs_bc = _partition_stats(
                nc, (ph, spps), [tok[:, cc, :] for cc in range(2)], S, 2,
                eps1, ones_sb, "full")
            kn = ph.tile([128, 2, S], PROJ_DT)
            for cc in range(2):
                t0 = ph.tile([128, S], F32, tag="st_scr")
                nc.vector.tensor_sub(t0[:], tok[:, cc, :], mu_bc[:])
                nc.vector.tensor_mul(t0[:], t0[:], rs_bc[:])
                nc.scalar.activation(kn[:, cc, :], t0[:], AF.Identity,
                                     bias=cv[:, cc, CV_NKVB:CV_NKVB + 1], scale=cv[:, cc, CV_NKVG:CV_NKVG + 1])
            muq_bc, rsq_bc = _partition_stats(
                nc, (ph, spps), [tokq[:, cc, :] for cc in range(2)], NQ, 2,
                eps1, ones_sb, "q")
            for cc in range(2):
                t0 = ph.tile([128, NQ], F32, tag="st_scr")
                nc.vector.tensor_sub(t0[:], tokq[:, cc, :], muq_bc[:])
                nc.vector.tensor_mul(t0[:], t0[:], rsq_bc[:])
                nc.scalar.activation(qn_sb[:, cc, :], t0[:], AF.Identity,
                                     bias=cv[:, cc, CV_NQB:CV_NQB + 1], scale=cv[:, cc, CV_NQG:CV_NQG + 1])

            # ---- projections ----
            STILE = [512, 512, 512, 512, 256]
            kst = ph.tile([128, 2, S], SCORES_DT, tag="bigbuf1")    # reuses xk slot
            for dc in range(2):
                s0 = 0
                for stn in STILE:
                    kp = pps.tile([128, 512], F32, tag="projps")
                    for cc in range(2):
                        nc.tensor.matmul(kp[:, :stn],
                                         kw_sb[:, cc, dc * 128:(dc + 1) * 128],
                                         kn[:, cc, s0:s0 + stn],
                                         start=(cc == 0), stop=(cc == 1))
                    nc.scalar.activation(kst[:, dc, s0:s0 + stn], kp[:, :stn],
                                         AF.Identity, bias=dv[:, dc, 1:2])
                    s0 += stn
            for p in range(4):
                dc = p // 2
                b0 = 64 * (p % 2)
                for hh in range(2):
                    nc.sync.dma_start(out=ktd[p, :, hh, :],
                                      in_=kst[b0 + 32 * hh:b0 + 32 * hh + 32, dc, :])
            qst = ph.tile([128, 2, NQ], SCORES_DT, tag="bigbuf2")
            for dc in range(2):
                for (s0, stn) in ((0, 512), (512, 64)):
                    qp = pps.tile([128, 512], F32, tag="projps")
                    for cc in range(2):
                        nc.tensor.matmul(qp[:, :stn],
                                         qw_sb[:, cc, dc * 128:(dc + 1) * 128],
                                         qn_sb[:, cc, s0:s0 + stn],
                                         start=(cc == 0), stop=(cc == 1))
                    nc.scalar.activation(qst[:, dc, s0:s0 + stn], qp[:, :stn],
                                         AF.Identity, bias=dv[:, dc, 0:1])
            for p in range(4):
                dc = p // 2
                b0 = 64 * (p % 2)
                for hh in range(2):
                    nc.sync.dma_start(out=qtd[p, :, hh, :],
                                      in_=qst[b0 + 32 * hh:b0 + 32 * hh + 32, dc, :])
            # v (token-major) -- bias folded into o_b_eff on host
            nc.vector.tensor_copy(
                v_tok[:, :, :, 32:33],
                ones_sb[:, None, None, :].to_broadcast([128, 18, NH, 1]))
            for sc in range(18):
                vp = pps.tile([128, 512], F32, tag="projps")
                for cc in range(2):
                    kn_v = (kn[:, cc, sc * 128:(sc + 1) * 128]
                            if PROJ_DT == VPROJ_DT else
                            kn.bitcast(VPROJ_DT)[:, cc, sc * 128:(sc + 1) * 128])
                    nc.tensor.matmul(vp[:, 0:C], kn_v,
                                     vw_sb[:, cc, :], start=(cc == 0), stop=(cc == 1))
                nc.vector.tensor_copy(
                    v_tok[:, sc, :, 0:32],
                    vp[:, 0:C].rearrange("p (h d) -> p h d", d=32))
            # gate logits -> exp(-z)
            for ic in range(6):
                gp = pps.tile([MC, 1], F32, tag="mlpps")
                for cc in range(2):
                    nc.tensor.matmul(gp[:, :],
                                     qn_sb.bitcast(F32)[:, cc, ic * MC:(ic + 1) * MC],
                                     cv[:, cc, CV_GW:CV_GW + 1],
                                     start=(cc == 0), stop=(cc == 1))
                nc.scalar.activation(eg_sb[:, ic:ic + 1], gp[:, :], AF.Exp, scale=-1.0)
            nc.vector.tensor_scalar(gate_sb[:], eg_sb[:], 1.0, None, OP.add)
            nc.vector.reciprocal(gate_sb[:], gate_sb[:])

        # ================= attention ==================
        with tc.tile_pool(name="att", bufs=2) as att, \
             tc.tile_pool(name="atte", bufs=3) as atte, \
             tc.tile_pool(name="ps_s", bufs=2, space="PSUM") as ps_s, \
             tc.tile_pool(name="ps_av", bufs=1, space="PSUM") as ps_av, \
             tc.tile_pool(name="ps_o", bufs=2, space="PSUM") as ps_o:
            for p in range(4):
                ktp = att.tile([32, 2, S], SCORES_DT, tag="ktp")
                nc.sync.dma_start(out=ktp, in_=ktd[p, :, :, :])
                qtp = att.tile([32, 2, NQ], SCORES_DT, tag="qtp")
                nc.sync.dma_start(out=qtp, in_=qtd[p, :, :, :])
                for it in range(2):
                    i0 = it * IT
                    av_ps = ps_av.tile([33, 2, 512], F32, tag="avps")
                    for jc in range(18):
                        s_ps = ps_s.tile([128, 2, 512], F32, tag="sps")
                        for hh in range(2):
                            nc.tensor.matmul(
                                s_ps[:, hh, 0:IT],
                                ktp[:, hh, jc * 128:(jc + 1) * 128],
                                qtp[:, hh, i0:i0 + IT], start=True, stop=True)
                        e_sb = atte.tile([128, 2, IT], AV_DT, tag="esb")
                        nc.scalar.activation(e_sb[:, :, :], s_ps[:, :, 0:IT],
                                             AF.Exp, scale=SCALE)
                        for hh in range(2):
                            nc.tensor.matmul(
                                av_ps[:, hh, 0:IT], v_tok[:, jc, 2 * p + hh, :],
                                e_sb[:, hh, :], start=(jc == 0), stop=(jc == 17))
                    av_sb = att.tile([33, 2, IT], OPROJ_DT, tag="avsb")
                    nc.vector.tensor_copy(av_sb[:, :, :], av_ps[:, :, 0:IT])
                    for hh in range(2):
                        h = 2 * p + hh
                        for mc in range(3):
                            ch = it * 3 + mc
                            o_ps = ps_o.tile([MC, 258], F32, tag="ops")
                            nc.tensor.matmul(o_ps[:, :],
                                             av_sb[:, hh, mc * MC:(mc + 1) * MC],
                                             ow_sb[0:33, h, :], start=True, stop=True)
                            r_sb = atte.tile([MC, 1], F32, tag="rsb")
                            nc.vector.reciprocal(r_sb[:], o_ps[:, 256:257])
                            if h == 0:
                                nc.vector.tensor_scalar_mul(
                                    out_acc[:, ch, :], o_ps[:, 0:256], r_sb[:])
                            else:
                                nc.vector.affine_then_add(
                                    out_acc[:, ch, :], o_ps[:, 0:256],
                                    out_acc[:, ch, :], r_sb[:], 0.0)

        # ================= epilogue ==================
        with tc.tile_pool(name="ep", bufs=2) as ep:
            epsm = cons.tile([MC, 1], F32)
            nc.vector.memset(epsm[:], EPS)
            og_all = ep.tile([MC, 6, C], F32, tag="og")
            mv_all = ep.tile([MC, 6, 2], F32, tag="bag")
            for ch in range(6):
                nc.vector.tensor_add(og_all[:, ch, :], out_acc[:, ch, :],
                                     rv_sb[0:MC, RV_OB, :])
                nc.scalar.activation(og_all[:, ch, :], og_all[:, ch, :], AF.Identity,
                                     scale=gate_sb[:, ch:ch + 1])
                stats = ep.tile([MC, nc.vector.BN_STATS_DIM], F32, tag="bst")
                nc.vector.bn_stats(stats[:], og_all[:, ch, :])
                nc.vector.bn_aggr(mv_all[:, ch, :], stats[:])
            # one Ln + one Exp for all 6 chunk rsqrts (avoids table ping-pong)
            rs_all = ep.tile([MC, 6], F32, tag="eprs")
            nc.scalar.activation(rs_all[:], mv_all[:, :, 1], AF.Ln,
                                 bias=epsm[:], scale=1.0)
            nc.scalar.activation(rs_all[:], rs_all[:], AF.Exp, scale=-0.5)
            for ch in range(6):
                t2 = ep.tile([MC, C], F32, tag="ept2")
                nc.vector.tensor_scalar(t2[:], og_all[:, ch, :],
                                        mv_all[:, ch, 0:1], None, OP.subtract)
                nc.vector.tensor_scalar_mul(t2[:], t2[:], rs_all[:, ch:ch + 1])
                nc.vector.tensor_mul(t2[:], t2[:], rv_sb[0:MC, RV_NOG, :])
                nc.vector.tensor_add(t2[:], t2[:], rv_sb[0:MC, RV_NOB, :])
                nc.vector.tensor_add(t2[:], t2[:], xqres_sb[:, ch, :])
                nc.sync.dma_start(
                    out=y.rearrange("(k p) c -> p k c", p=MC)[:, ch, :], in_=t2[:])


def _host_inputs(x, text_feature, tm_w1, tm_b1, tm_ln1_g, tm_ln1_b, tm_w2, tm_b2,
                 tm_ln2_g, tm_ln2_b, conv_w, conv_b, q_w, q_b, k_w, k_b, v_w, v_b,
                 o_w, o_b, gate_w, nq_g, nq_b, nkv_g, nkv_b, no_g, no_b):
    f = np.float32
    # pe table (depends only on (c, w); faithful to reference)
    div = np.exp(np.arange(C // 2, dtype=f) * (-math.log(10000.0) / (C // 2)))
    wpos = np.arange(W, dtype=f)
    s = np.sin(wpos[None, :] * div[:, None])
    c = np.cos(wpos[None, :] * div[:, None])
    pe = np.stack([s, c], axis=1).reshape(C, W).astype(f)
    # kh-collapsed conv kernels: top(kh 1,2), mid(all), bot(kh 0,1)
    w3 = np.stack([
        conv_w[:, :, 1, :] + conv_w[:, :, 2, :],
        conv_w.sum(axis=2),
        conv_w[:, :, 0, :] + conv_w[:, :, 1, :],
    ]).astype(f)                                  # [3, Cout, Cin, kw]
    w3 = w3.transpose(0, 3, 2, 1).reshape(3, 768, C)  # [(kw, cin), cout]
    w3 = np.ascontiguousarray(w3, dtype=f)
    cvecs = np.stack([
        tm_b1, tm_ln1_g, tm_ln1_b, tm_b2, -tm_ln2_g, -tm_ln2_b,
        nq_g, nq_b, nkv_g, nkv_b, conv_b, gate_w[0],
    ], axis=1).astype(f)                          # [256, 12]
    dvecs = np.stack([q_b, k_b], axis=1).astype(f)
    owx = np.zeros((128, NH, 258), f)
    for h in range(NH):
        owx[0:32, h, 0:256] = o_w[:, 32 * h:32 * h + 32].T
        owx[32, h, 256] = 1.0
    ob_eff = (o_b + v_b @ o_w.T).astype(f)
    rowvecs = np.broadcast_to(
        np.stack([ob_eff, no_g, no_b])[None, :, :], (128, 3, C)).astype(f)
    rowvecs = np.ascontiguousarray(rowvecs)

    per_core = []
    for core in range(8):
        b, k = core // 4, core % 4
        xb = np.ascontiguousarray(x[b].reshape(C, S), dtype=f)
        xqc = np.ascontiguousarray(xb[:, NQ * k:NQ * (k + 1)])
        sel = np.zeros((128, 2, NQ), f)
        if k == 0:
            sel[:, 0, 0:W] = 1.0
        if k == 3:
            sel[:, 1, NQ - W:NQ] = 1.0
        per_core.append({
            "xk": xb,
            "xq": xqc,
            "xqres": np.ascontiguousarray(xqc.T),
            "text": np.ascontiguousarray(text_feature[b][:, None], dtype=f),
            "tmw1": np.ascontiguousarray(tm_w1.T, dtype=f),
            "tmw2": np.ascontiguousarray(tm_w2.T, dtype=f),
            "cvecs": cvecs, "dvecs": dvecs, "pe": pe, "w3": w3,
            "qwT": np.ascontiguousarray(q_w.T, dtype=f),
            "kwT": np.ascontiguousarray(k_w.T, dtype=f),
            "vwT": np.ascontiguousarray(v_w.T, dtype=f),
            "owx": owx, "rowvecs": rowvecs, "selmask": sel,
        })
    return per_core


_NC_CACHE = {}


def get_nc():
    if "nc" not in _NC_CACHE:
        _NC_CACHE["nc"] = build_bass()
    return _NC_CACHE["nc"]


def kernel(**inputs):
    inputs = {k: np.asarray(v, dtype=np.float32) for k, v in inputs.items()}
    in_maps = _host_inputs(**inputs)
    nc = get_nc()
    res = run_bass_kernel_spmd(nc, in_maps, core_ids=list(range(8)))
    x = inputs["x"]
    out = np.empty((B, C, H, W), np.float32)
    for b in range(B):
        blocks = [res.results[4 * b + k]["y"] for k in range(4)]  # [NQ, C] each
        tok = np.concatenate(blocks, axis=0)                      # [S, C]
        out[b] = tok.T.reshape(C, H, W)
    return out



# revision 5
# speedup vs baseline: 1.1567x; 1.1567x over previous
"""Trainium2 Bass kernel for nn_EnhancedTextAttentionBlock.

Self-contained: takes FULL inputs (as in reference.setup_inputs()), shards
across 8 NeuronCores internally, returns the FULL [2, 256, 48, 48] output.

Sharding: core c handles batch b = c // 4 and query-token block k = c % 4
(576 of the 2304 spatial tokens). K/V are computed for the full token set on
every core; a single SPMD program serves all 8 cores with no collectives.

Key structure (all exact algebraic restructurings, except the fp32->f32r
matmul dtype and a Newton-refined inverse-sqrt, both far inside the error
budget):
- pe depends only on (c, w): the 3x3 conv collapses to 3 distinct rows
  (top/mid/bottom) computed as small matmuls.
- LN gains/biases of nq/nkv are folded into the q/k/v projection weights on
  the host; the kernel only applies the (x - mu) * rsqrt(var) part.
- rsqrt everywhere via DVE integer fast-inverse-sqrt + 3 Newton steps: the
  kernel then needs only {Exp, Identity, Relu, Square} activations, which
  live in ONE activation table (no table ping-pong).
- LN statistics are computed with ones-matmuls, then repacked [1,S] ->
  [128,S/128] by DMA so the scalar math runs 128-wide.
- Scores read the c-major K/Q projections directly as 32-partition slices
  (PE tile_position); heads at partition offset 96 are staged through a
  small SBUF->SBUF DMA (offsets are limited to 0/32/64) and scheduled last.
- v carries a ones-column so softmax denominators l ride through the AV
  matmul; av is normalized by 1/l BEFORE the out-projection, so the 8 heads
  accumulate in PSUM and o_b (+ v_b @ o_w.T) folds into a 33rd row of the
  out-projection weights.
- Softmax max-subtraction is skipped: LN'd activations through 0.02-scale
  weights keep |scores| small enough for exact fp32 exp.
"""
import math
import numpy as np

import concourse.bass as bass
import concourse.tile as tile
from concourse import bacc, mybir
from concourse.bass_utils import run_bass_kernel_spmd

F32 = mybir.dt.float32
F32R = mybir.dt.float32r
I32 = mybir.dt.int32
AF = mybir.ActivationFunctionType
OP = mybir.AluOpType

B, C, H, W, T = 2, 256, 48, 48, 512
NH, HD = 8, 32
S = H * W              # 2304 tokens
NQ = S // 4            # 576 q tokens per core
SCALE = HD ** -0.5
IT = 288               # q block (two per core)
MC = 96                # epilogue chunk
EPS = 1e-5
MAGIC = 0x5F3759DF

# cvecs column indices (c-major [256, 1] vectors packed into one input)
CV_TMB1, CV_L1G, CV_L1B, CV_TMB2, CV_L2GN, CV_L2BN, CV_CONVB, CV_GWG = range(8)

# head h -> 32-channel slice of the c-major projections: chunk dc = h // 4,
# partition offset 32*(h%4).  Offset-96 heads (3, 7) are staged to kst3/qst3.
PAIRS = [(0, 1), (2, 4), (5, 6), (3, 7)]


def _fisr(nc, pool, x_ap, pshape, tag, iters=3):
    """rsqrt(x) on DVE: int bit-trick seed + Newton. x_ap: SBUF f32 AP > 0.
    Returns an F32 AP of a fresh tile."""
    P, Fn = pshape
    sh = pool.tile([P, Fn], I32, tag=f"{tag}_i")
    nc.vector.tensor_scalar(sh[:], x_ap.bitcast(I32), 1, None,
                            OP.logical_shift_right)
    nc.vector.tensor_scalar(sh[:], sh[:], -1, None, OP.bitwise_xor)
    nc.vector.tensor_scalar(sh[:], sh[:], MAGIC + 1, None, OP.add)
    y = sh.bitcast(F32)
    t = pool.tile([P, Fn], F32, tag=f"{tag}_t")
    for _ in range(iters):
        nc.vector.tensor_mul(t[:], y[:], y[:])
        nc.vector.tensor_tensor(t[:], t[:], x_ap, OP.mult)
        nc.vector.tensor_scalar(t[:], t[:], -0.5, 1.5, OP.mult, OP.add)
        nc.vector.tensor_mul(y[:], y[:], t[:])
    return y


def build_bass():
    nc = bacc.Bacc("TRN2", target_bir_lowering=False, debug=False,
                   enable_asserts=True, num_devices=8)
    di = {}

    def inp(name, shape, dt=F32):
        di[name] = nc.dram_tensor(name, shape, dt, kind="ExternalInput")
        return di[name]

    inp("xk", [C, S])
    inp("xq", [C, NQ])
    inp("xqres2", [NQ, C])          # xq residual (token-major) + no_b
    inp("text", [T, 1])
    inp("tmw1", [T, C])
    inp("tmw2", [C, C])
    inp("cvecs", [C, 8])
    inp("dvecs", [C, 2])            # qb_fold, kb_fold
    inp("pe", [C, W])
    inp("w3", [3, 768, C])
    inp("qwT", [C, C], F32R)        # (q_w * nq_g).T
    inp("kwT", [C, C], F32R)        # (k_w * nkv_g).T
    inp("vwT", [C, C], F32R)        # (v_w * nkv_g).T
    inp("ow2", [33, NH, C], F32R)   # per-head o_w rows + ob_eff/8 row
    inp("nogr", [1, C])             # no_g row
    inp("selmask", [128, 2, W])     # top/bottom q-edge masks for this core
    inp("gbneg", [MC, 1])           # -(gate_w @ nq_b), prebroadcast
    y = nc.dram_tensor("y", [NQ, C], F32, kind="ExternalOutput")

    with tile.TileContext(nc) as tc:
        _build_tile(nc, tc, di, y)
    nc.compile()
    return nc


def _build_tile(nc, tc, di, y):
    with tc.tile_pool(name="cons", bufs=1) as cons:
        # ---- persistent tiles ----
        ones_sb = cons.tile([128, 1], F32R)
        nc.vector.memset(ones_sb[:], 1.0)
        cv = cons.tile([128, 2, 8], F32)
        nc.sync.dma_start(out=cv, in_=di["cvecs"].rearrange("(c p) v -> p c v", p=128))
        dv = cons.tile([128, 2, 2], F32)
        nc.sync.dma_start(out=dv, in_=di["dvecs"].rearrange("(c p) v -> p c v", p=128))
        pe_sb = cons.tile([128, 2, W], F32)
        nc.sync.dma_start(out=pe_sb, in_=di["pe"].rearrange("(c p) w -> p c w", p=128))
        qw_sb = cons.tile([128, 2, C], F32R)
        kw_sb = cons.tile([128, 2, C], F32R)
        vw_sb = cons.tile([128, 2, C], F32R)
        ow_sb = cons.tile([33, NH, C], F32R)
        nogr_sb = cons.tile([1, C], F32)
        sel_sb = cons.tile([128, 2, W], F32)
        gb_sb = cons.tile([MC, 1], F32)
        posrow = cons.tile([128, 2, 3, W], F32)   # (cc, rowtype, w)
        dtop = cons.tile([128, 2, W], F32)
        dbot = cons.tile([128, 2, W], F32)
        kst = cons.tile([128, 2, S], F32R)
        kst3 = cons.tile([32, 2, S], F32R)        # heads 3, 7
        qst = cons.tile([128, 2, NQ], F32R)
        qst3 = cons.tile([32, 2, NQ], F32R)
        v_tok = cons.tile([128, 18, NH, 33], F32R)
        avn_all = cons.tile([33, NH, NQ], F32R)
        gate_sb = cons.tile([MC, 6], F32)
        xqres_sb = cons.tile([MC, 6, C], F32)

        # ================= prologue ==================
        with tc.tile_pool(name="ph", bufs=1) as ph, \
             tc.tile_pool(name="pps", bufs=2, space="PSUM") as pps, \
             tc.tile_pool(name="spps", bufs=1, space="PSUM") as spps:
            # critical-path DMAs first
            xk_sb = ph.tile([128, 2, S], F32, tag="phA")
            nc.sync.dma_start(out=xk_sb,
                              in_=di["xk"].rearrange("(c p) s -> p c s", p=128))
            text_sb = ph.tile([128, 4, 1], F32)
            nc.sync.dma_start(out=text_sb,
                              in_=di["text"].rearrange("(k p) o -> p k o", p=128))
            w1_sb = ph.tile([128, 4, C], F32)
            nc.sync.dma_start(out=w1_sb,
                              in_=di["tmw1"].rearrange("(k p) d -> p k d", p=128))
            w2_sb = ph.tile([128, 2, C], F32)
            nc.sync.dma_start(out=w2_sb,
                              in_=di["tmw2"].rearrange("(k p) d -> p k d", p=128))
            w3_sb = ph.tile([128, 3, 6, C], F32, tag="phC")
            nc.sync.dma_start(out=w3_sb,
                              in_=di["w3"].rearrange("t (j p) m -> p t j m", p=128))
            xq_sb = ph.tile([128, 2, NQ], F32)
            nc.sync.dma_start(out=xq_sb,
                              in_=di["xq"].rearrange("(c p) s -> p c s", p=128))
            nc.sync.dma_start(out=sel_sb, in_=di["selmask"][:, :, :])
            # non-critical loads
            nc.sync.dma_start(out=qw_sb, in_=di["qwT"].rearrange("(c p) d -> p c d", p=128))
            nc.sync.dma_start(out=kw_sb, in_=di["kwT"].rearrange("(c p) d -> p c d", p=128))
            nc.sync.dma_start(out=vw_sb, in_=di["vwT"].rearrange("(c p) d -> p c d", p=128))
            nc.sync.dma_start(out=ow_sb, in_=di["ow2"][:, :, :])
            nc.sync.dma_start(out=nogr_sb, in_=di["nogr"][:, :])
            nc.sync.dma_start(out=gb_sb, in_=di["gbneg"][:, :])
            nc.sync.dma_start(out=xqres_sb,
                              in_=di["xqres2"].rearrange("(k p) c -> p k c", p=MC))

            # ---- text modulation MLP (c-major) ----
            def cmajor_mlp_layer(x_col, w_sb, nkc, bias_col, tag):
                h_col = ph.tile([128, 2, 1], F32, tag=f"{tag}_h")
                for c2c in range(2):
                    h_ps = pps.tile([128, 1], F32, tag="mlpps")
                    for kc in range(nkc):
                        nc.tensor.matmul(
                            h_ps[:, :], w_sb[:, kc, c2c * 128:(c2c + 1) * 128],
                            x_col[:, kc, :], start=(kc == 0), stop=(kc == nkc - 1))
                    nc.scalar.activation(h_col[:, c2c, :], h_ps[:, :], AF.Identity,
                                         bias=bias_col[:, c2c, :])
                return h_col

            def cmajor_ln_rs(h_col, tag):
                # 256-dim stats of [128, 2, 1] -> broadcast [128,1] rs, murs
                sum_ps = spps.tile([1, 1], F32, tag="ssum")
                sq_ps = spps.tile([1, 1], F32, tag="ssq")
                hsq = ph.tile([128, 2, 1], F32R, tag=f"{tag}_hsq")
                nc.scalar.activation(hsq[:], h_col[:], AF.Square)
                for cc in range(2):
                    nc.tensor.matmul(sum_ps[:, :], ones_sb[:],
                                     h_col.bitcast(F32R)[:, cc, :],
                                     start=(cc == 0), stop=(cc == 1))
                    nc.tensor.matmul(sq_ps[:, :], ones_sb[:], hsq[:, cc, :],
                                     start=(cc == 0), stop=(cc == 1))
                mu1 = ph.tile([1, 2], F32, tag=f"{tag}_mu1")
                nc.vector.tensor_scalar_mul(mu1[:, 0:1], sum_ps[:, :], 1.0 / 256.0)
                nc.vector.tensor_scalar_mul(mu1[:, 1:2], sq_ps[:, :], 1.0 / 256.0)
                var1 = ph.tile([1, 1], F32, tag=f"{tag}_var1")
                nc.vector.tensor_mul(var1[:], mu1[:, 0:1], mu1[:, 0:1])
                nc.vector.tensor_tensor(var1[:], mu1[:, 1:2], var1[:], OP.subtract)
                nc.vector.tensor_scalar(var1[:], var1[:], EPS, None, OP.add)
                rs1 = _fisr(nc, ph, var1[:], (1, 1), f"{tag}_f")
                murs1 = ph.tile([1, 1], F32, tag=f"{tag}_mrs")
                nc.vector.tensor_tensor(murs1[:], mu1[:, 0:1], rs1[:], OP.mult)
                rs_b = ph.tile([128, 1], F32, tag=f"{tag}_rsb")
                nc.gpsimd.partition_broadcast(rs_b[:], rs1[:])
                murs_b = ph.tile([128, 1], F32, tag=f"{tag}_mub")
                nc.gpsimd.partition_broadcast(murs_b[:], murs1[:])
                return rs_b, murs_b

            h1 = cmajor_mlp_layer(text_sb, w1_sb, 4, cv[:, :, CV_TMB1:CV_TMB1 + 1], "l1")
            rs_b, murs_b = cmajor_ln_rs(h1, "l1")
            h1n = ph.tile([128, 2, 1], F32, tag="h1n")
            mod = ph.tile([128, 2, 1], F32, tag="mod")
            for cc in range(2):
                nc.vector.tensor_scalar(h1n[:, cc, :], h1[:, cc, :], rs_b[:],
                                        murs_b[:], OP.mult, OP.subtract)
                nc.scalar.activation(h1n[:, cc, :], h1n[:, cc, :], AF.Relu,
                                     bias=cv[:, cc, CV_L1B:CV_L1B + 1],
                                     scale=cv[:, cc, CV_L1G:CV_L1G + 1])
            h2 = cmajor_mlp_layer(h1n, w2_sb, 2, cv[:, :, CV_TMB2:CV_TMB2 + 1], "l2")
            rs2_b, murs2_b = cmajor_ln_rs(h2, "l2")
            for cc in range(2):
                nc.vector.tensor_scalar(mod[:, cc, :], h2[:, cc, :], rs2_b[:],
                                        murs2_b[:], OP.mult, OP.subtract)
                # sigmoid(z) = 1/(1 + exp(-(g*xn+b))) via pre-negated g, b
                nc.scalar.activation(mod[:, cc, :], mod[:, cc, :], AF.Exp,
                                     bias=cv[:, cc, CV_L2BN:CV_L2BN + 1],
                                     scale=cv[:, cc, CV_L2GN:CV_L2GN + 1])
                nc.vector.tensor_scalar(mod[:, cc, :], mod[:, cc, :], 1.0, None, OP.add)
                nc.vector.reciprocal(mod[:, cc, :], mod[:, cc, :])

            # ---- conditional positional rows: 3 distinct conv rows ----
            inrow = ph.tile([128, 2, W], F32)
            for cc in range(2):
                nc.vector.tensor_scalar_mul(inrow[:, cc, :], pe_sb[:, cc, :],
                                            mod[:, cc, 0:1])
            im2 = ph.tile([128, 6, W], F32)
            nc.vector.memset(im2[:], 0.0)
            for kw in range(3):
                for cc in range(2):
                    j = kw * 2 + cc
                    if kw == 0:
                        nc.vector.tensor_copy(im2[:, j, 1:W], inrow[:, cc, 0:W - 1])
                    elif kw == 1:
                        nc.vector.tensor_copy(im2[:, j, :], inrow[:, cc, :])
                    else:
                        nc.vector.tensor_copy(im2[:, j, 0:W - 1], inrow[:, cc, 1:W])
            cps = pps.tile([128, 3, 2, W], F32, tag="projps")
            for t in range(3):
                for oc in range(2):
                    for j in range(6):
                        nc.tensor.matmul(cps[:, t, oc, :],
                                         w3_sb.bitcast(F32R)[:, t, j, oc * 128:(oc + 1) * 128],
                                         im2.bitcast(F32R)[:, j, :],
                                         start=(j == 0), stop=(j == 5))
            for cc in range(2):
                nc.scalar.activation(posrow[:, cc, :, :], cps[:, :, cc, :], AF.Identity,
                                     bias=cv[:, cc, CV_CONVB:CV_CONVB + 1])
                nc.vector.tensor_sub(dtop[:, cc, :], posrow[:, cc, 0, :],
                                     posrow[:, cc, 1, :])
                nc.vector.tensor_sub(dbot[:, cc, :], posrow[:, cc, 2, :],
                                     posrow[:, cc, 1, :])

            # ---- tokens (c-major); big mid-row adds split DVE / Pool ----
            tok = ph.tile([128, 2, S], F32, tag="phB")
            for cc in range(2):
                eng = nc.vector if cc == 0 else nc.gpsimd
                eng.tensor_add(tok[:, cc, 0:W], xk_sb[:, cc, 0:W],
                               posrow[:, cc, 0, :])
                mid = posrow[:, cc, 1:2, :].to_broadcast([128, H - 2, W])
                eng.tensor_tensor(
                    tok[:, cc, W:S - W].rearrange("p (h w) -> p h w", w=W),
                    xk_sb[:, cc, W:S - W].rearrange("p (h w) -> p h w", w=W),
                    mid, OP.add)
                eng.tensor_add(tok[:, cc, S - W:S], xk_sb[:, cc, S - W:S],
                               posrow[:, cc, 2, :])
            tokq = ph.tile([128, 2, NQ], F32)
            edge = ph.tile([128, W], F32, tag="edge")
            for cc in range(2):
                mid = posrow[:, cc, 1:2, :].to_broadcast([128, NQ // W, W])
                nc.vector.tensor_tensor(
                    tokq[:, cc, :].rearrange("p (h w) -> p h w", w=W),
                    xq_sb[:, cc, :].rearrange("p (h w) -> p h w", w=W),
                    mid, OP.add)
                nc.vector.tensor_mul(edge[:], sel_sb[:, 0, :], dtop[:, cc, :])
                nc.vector.tensor_add(tokq[:, cc, 0:W], tokq[:, cc, 0:W], edge[:])
                nc.vector.tensor_mul(edge[:], sel_sb[:, 1, :], dbot[:, cc, :])
                nc.vector.tensor_add(tokq[:, cc, NQ - W:NQ], tokq[:, cc, NQ - W:NQ],
                                     edge[:])

            # ---- LN stats: ones-matmul sums, packed [128, n] scalar math ----
            def ln_stats_rows(x_t, n_free, npk, tag, bc):
                """x_t: [128, 2, n_free] f32. Writes rs/murs rows into
                bc[0:1, 0/1, :] (partition 0 of the broadcast dest).
                npk = packed columns (4 per 512-chunk)."""
                pk = ph.tile([128, 2, npk], F32, tag=f"{tag}_pk")
                nc.vector.memset(pk[:], 1.0)
                nhalf = (n_free + 511) // 512
                for hf in range(nhalf):
                    f0 = hf * 512
                    fn = min(512, n_free - f0)
                    sum_ps = spps.tile([1, 512], F32, tag="stsum")
                    sq_ps = spps.tile([1, 512], F32, tag="stsq")
                    for cc in range(2):
                        sq = ph.tile([128, 512], F32R, tag=f"sqc{cc}")
                        nc.scalar.activation(sq[:, :fn], x_t[:, cc, f0:f0 + fn],
                                             AF.Square)
                        nc.tensor.matmul(sum_ps[:, :fn], ones_sb[:],
                                         x_t.bitcast(F32R)[:, cc, f0:f0 + fn],
                                         start=(cc == 0), stop=(cc == 1))
                        nc.tensor.matmul(sq_ps[:, :fn], ones_sb[:], sq[:, :fn],
                                         start=(cc == 0), stop=(cc == 1))
                    # stage PSUM rows to SBUF (DMA cannot read PSUM), then
                    # pack t = f0 + 4p + j  ->  pk[p, 4*hf + j]
                    stage = ph.tile([1, 2, 512], F32, tag=f"stg{hf % 2}")
                    nc.scalar.activation(stage[:, 0, 0:fn], sum_ps[:, 0:fn],
                                         AF.Identity)
                    nc.gpsimd.tensor_copy(stage[:, 1, 0:fn], sq_ps[:, 0:fn])
                    np_rows = fn // 4
                    nc.sync.dma_start(
                        out=pk[0:np_rows, 0, 4 * hf:4 * hf + 4],
                        in_=stage[0:1, 0, 0:fn].rearrange("o (p j) -> o p j", j=4))
                    nc.sync.dma_start(
                        out=pk[0:np_rows, 1, 4 * hf:4 * hf + 4],
                        in_=stage[0:1, 1, 0:fn].rearrange("o (p j) -> o p j", j=4))
                # packed math: mu = s/256, ex2 = sq/256, var = ex2 - mu^2
                m = ph.tile([128, 2, npk], F32, tag=f"{tag}_m")
                nc.vector.tensor_scalar_mul(m[:], pk[:], 1.0 / 256.0)
                varx = ph.tile([128, npk], F32, tag=f"{tag}_v")
                nc.vector.tensor_mul(varx[:], m[:, 0, :], m[:, 0, :])
                nc.vector.tensor_tensor(varx[:], m[:, 1, :], varx[:], OP.subtract)
                nc.vector.tensor_scalar(varx[:], varx[:], EPS, None, OP.add)
                rs_pk = _fisr(nc, ph, varx[:], (128, npk), f"{tag}_f")
                murs_pk = ph.tile([128, npk], F32, tag=f"{tag}_ms")
                nc.vector.tensor_tensor(murs_pk[:], m[:, 0, :], rs_pk[:], OP.mult)
                for hf in range(nhalf):
                    f0 = hf * 512
                    fn = min(512, n_free - f0)
                    np_rows = fn // 4
                    nc.sync.dma_start(
                        out=bc[0:1, 0, f0:f0 + fn].rearrange("o (p j) -> o p j", j=4),
                        in_=rs_pk[0:np_rows, 4 * hf:4 * hf + 4])
                    nc.sync.dma_start(
                        out=bc[0:1, 1, f0:f0 + fn].rearrange("o (p j) -> o p j", j=4),
                        in_=murs_pk[0:np_rows, 4 * hf:4 * hf + 4])

            # rs/murs rows land on partition 0 of bc tiles; broadcast in place
            bc_k = ph.tile([128, 2, S], F32, tag="phC")          # reuses w3 slot
            bc_q = ph.tile([128, 2, NQ], F32, tag="bcq")
            ln_stats_rows(tok, S, 20, "sk", bc_k)
            ln_stats_rows(tokq, NQ, 8, "sq", bc_q)
            for cx in range(2):
                nc.gpsimd.partition_broadcast(bc_k[:, cx, :], bc_k[0:1, cx, :])
                nc.gpsimd.partition_broadcast(bc_q[:, cx, :], bc_q[0:1, cx, :])
            kn = ph.tile([128, 2, S], F32R, tag="phA")           # reuses xk slot
            knf = kn.bitcast(F32)
            for cc in range(2):
                eng = nc.vector if cc == 0 else nc.gpsimd
                eng.tensor_tensor(knf[:, cc, :], tok[:, cc, :], bc_k[:, 0, :], OP.mult)
                eng.tensor_tensor(knf[:, cc, :], knf[:, cc, :], bc_k[:, 1, :],
                                  OP.subtract)
            qn = ph.tile([128, 2, NQ], F32R, tag="xq_sb")  # xq dead after tokq
            qnf = qn.bitcast(F32)
            for cc in range(2):
                nc.vector.tensor_tensor(qnf[:, cc, :], tokq[:, cc, :], bc_q[:, 0, :],
                                        OP.mult)
                nc.vector.tensor_tensor(qnf[:, cc, :], qnf[:, cc, :], bc_q[:, 1, :],
                                        OP.subtract)

            # ---- projections (all f32r) ----
            STILE = [512, 512, 512, 512, 256]
            for dc in range(2):
                s0 = 0
                for stn in STILE:
                    kp = pps.tile([128, 512], F32, tag="projps")
                    for cc in range(2):
                        nc.tensor.matmul(kp[:, :stn],
                                         kw_sb[:, cc, dc * 128:(dc + 1) * 128],
                                         kn[:, cc, s0:s0 + stn],
                                         start=(cc == 0), stop=(cc == 1))
                    nc.scalar.activation(kst[:, dc, s0:s0 + stn], kp[:, :stn],
                                         AF.Identity, bias=dv[:, dc, 1:2])
                    s0 += stn
            for dc in range(2):
                for (s0, stn) in ((0, 512), (512, 64)):
                    qp = pps.tile([128, 512], F32, tag="projps")
                    for cc in range(2):
                        nc.tensor.matmul(qp[:, :stn],
                                         qw_sb[:, cc, dc * 128:(dc + 1) * 128],
                                         qn[:, cc, s0:s0 + stn],
                                         start=(cc == 0), stop=(cc == 1))
                    nc.scalar.activation(qst[:, dc, s0:s0 + stn], qp[:, :stn],
                                         AF.Identity, bias=dv[:, dc, 0:1])
            # stage offset-96 heads (3, 7) to partition-0 tiles
            nc.sync.dma_start(out=kst3[:, :, :], in_=kst[96:128, :, :])
            nc.sync.dma_start(out=qst3[:, :, :], in_=qst[96:128, :, :])
            # v (token-major); v_b folded into ow2's ob_eff row
            nc.vector.tensor_copy(
                v_tok[:, :, :, 32:33],
                ones_sb[:, None, None, :].to_broadcast([128, 18, NH, 1]))
            for sc in range(18):
                vp = pps.tile([128, 512], F32, tag="projps")
                for cc in range(2):
                    nc.tensor.matmul(vp[:, 0:C], kn[:, cc, sc * 128:(sc + 1) * 128],
                                     vw_sb[:, cc, :], start=(cc == 0), stop=(cc == 1))
                eng = nc.vector if sc % 2 == 0 else nc.gpsimd
                eng.tensor_copy(
                    v_tok[:, sc, :, 0:32],
                    vp[:, 0:C].rearrange("p (h d) -> p h d", d=32))
            # gate logits -> exp(-(z + gb))
            eg_sb = ph.tile([MC, 6], F32, tag="eg")
            for ic in range(6):
                gp = pps.tile([MC, 1], F32, tag="mlpps")
                for cc in range(2):
                    nc.tensor.matmul(gp[:, :],
                                     qn[:, cc, ic * MC:(ic + 1) * MC],
                                     cv.bitcast(F32R)[:, cc, CV_GWG:CV_GWG + 1],
                                     start=(cc == 0), stop=(cc == 1))
                nc.scalar.activation(eg_sb[:, ic:ic + 1], gp[:, :], AF.Exp,
                                     scale=-1.0, bias=gb_sb[:, :])
            nc.vector.tensor_scalar(gate_sb[:], eg_sb[:], 1.0, None, OP.add)
            nc.vector.reciprocal(gate_sb[:], gate_sb[:])

        # ================= attention ==================
        def kslc(h, jc):
            if h == 3 or h == 7:
                return kst3[:, h // 4, jc * 128:(jc + 1) * 128]
            return kst[32 * (h % 4):32 * (h % 4) + 32, h // 4,
                       jc * 128:(jc + 1) * 128]

        def qslc(h, it):
            if h == 3 or h == 7:
                return qst3[:, h // 4, it * IT:(it + 1) * IT]
            return qst[32 * (h % 4):32 * (h % 4) + 32, h // 4,
                       it * IT:(it + 1) * IT]

        with tc.tile_pool(name="atte", bufs=3) as atte, \
             tc.tile_pool(name="ps_s", bufs=2, space="PSUM") as ps_s, \
             tc.tile_pool(name="ps_av", bufs=2, space="PSUM") as ps_av:
            for (hA, hB) in PAIRS:
                for it in range(2):
                    av_ps = ps_av.tile([33, 2, 512], F32, tag="avps")
                    prev = None

                    def emit_av(e_jc):
                        e_sb, jc = e_jc
                        for hh, h in enumerate((hA, hB)):
                            nc.tensor.matmul(
                                av_ps[:, hh, 0:IT], v_tok[:, jc, h, :],
                                e_sb[:, hh, :], start=(jc == 0), stop=(jc == 17))

                    for jc in range(18):
                        s_ps = ps_s.tile([128, 2, 512], F32, tag="sps")
                        for hh, h in enumerate((hA, hB)):
                            nc.tensor.matmul(s_ps[:, hh, 0:IT], kslc(h, jc),
                                             qslc(h, it), start=True, stop=True)
                        e_sb = atte.tile([128, 2, IT], F32R, tag="esb")
                        nc.scalar.activation(e_sb[:, :, :], s_ps[:, :, 0:IT],
                                             AF.Exp, scale=SCALE)
                        if prev is not None:
                            emit_av(prev)
                        prev = (e_sb, jc)
                    emit_av(prev)
                    # normalize by 1/l (row 32 of av_ps) into avn_all
                    r1 = atte.tile([1, 2, IT], F32, tag="rsb")
                    nc.vector.reciprocal(r1[:], av_ps[32:33, :, 0:IT])
                    rb = atte.tile([33, 2, IT], F32, tag="rbb")
                    nc.gpsimd.partition_broadcast(rb[:, 0, :], r1[0:1, 0, :])
                    nc.gpsimd.partition_broadcast(rb[:, 1, :], r1[0:1, 1, :])
                    for hh, h in enumerate((hA, hB)):
                        nc.vector.tensor_tensor(
                            avn_all.bitcast(F32)[:, h, it * IT:(it + 1) * IT],
                            av_ps[:, hh, 0:IT], rb[:, hh, :], OP.mult)

        # ================= out-projection + epilogue ==================
        with tc.tile_pool(name="ep", bufs=2) as ep, \
             tc.tile_pool(name="epc", bufs=1) as epc, \
             tc.tile_pool(name="ps_o", bufs=3, space="PSUM") as ps_o:
            og_all = epc.tile([MC, 6, C], F32)
            mv_all = epc.tile([MC, 6, 2], F32)
            for ch in range(6):
                o_ps = ps_o.tile([MC, C], F32, tag="ops")
                for h in range(NH):
                    nc.tensor.matmul(o_ps[:, :],
                                     avn_all[:, h, ch * MC:(ch + 1) * MC],
                                     ow_sb[:, h, :], start=(h == 0), stop=(h == 7))
                nc.scalar.activation(og_all[:, ch, :], o_ps[:, :], AF.Identity,
                                     scale=gate_sb[:, ch:ch + 1])
                stats = ep.tile([MC, nc.vector.BN_STATS_DIM], F32, tag="bst")
                nc.vector.bn_stats(stats[:], og_all[:, ch, :])
                nc.vector.bn_aggr(mv_all[:, ch, :], stats[:])
            varx = epc.tile([MC, 6], F32)
            nc.vector.tensor_scalar(varx[:], mv_all[:, :, 1], EPS, None, OP.add)
            rs_all = _fisr(nc, epc, varx[:], (MC, 6), "ef")
            murs_all = epc.tile([MC, 6], F32)
            nc.vector.tensor_tensor(murs_all[:], mv_all[:, :, 0], rs_all[:], OP.mult)
            nogb = epc.tile([MC, C], F32)
            nc.gpsimd.partition_broadcast(nogb[:], nogr_sb[0:1, :])
            for ch in range(6):
                t2 = ep.tile([MC, C], F32, tag="ept2")
                nc.vector.tensor_scalar(t2[:], og_all[:, ch, :],
                                        rs_all[:, ch:ch + 1],
                                        murs_all[:, ch:ch + 1],
                                        OP.mult, OP.subtract)
                nc.vector.tensor_mul(t2[:], t2[:], nogb[:])
                nc.vector.tensor_add(t2[:], t2[:], xqres_sb[:, ch, :])
                nc.sync.dma_start(
                    out=y.rearrange("(k p) c -> p k c", p=MC)[:, ch, :], in_=t2[:])


def _host_inputs(x, text_feature, tm_w1, tm_b1, tm_ln1_g, tm_ln1_b, tm_w2, tm_b2,
                 tm_ln2_g, tm_ln2_b, conv_w, conv_b, q_w, q_b, k_w, k_b, v_w, v_b,
                 o_w, o_b, gate_w, nq_g, nq_b, nkv_g, nkv_b, no_g, no_b):
    f = np.float32
    # pe table (depends only on (c, w); faithful to reference)
    div = np.exp(np.arange(C // 2, dtype=f) * (-math.log(10000.0) / (C // 2)))
    wpos = np.arange(W, dtype=f)
    s = np.sin(wpos[None, :] * div[:, None])
    c = np.cos(wpos[None, :] * div[:, None])
    pe = np.stack([s, c], axis=1).reshape(C, W).astype(f)
    # kh-collapsed conv kernels: top(kh 1,2), mid(all), bot(kh 0,1)
    w3 = np.stack([
        conv_w[:, :, 1, :] + conv_w[:, :, 2, :],
        conv_w.sum(axis=2),
        conv_w[:, :, 0, :] + conv_w[:, :, 1, :],
    ]).astype(f)                                  # [3, Cout, Cin, kw]
    w3 = w3.transpose(0, 3, 2, 1).reshape(3, 768, C)  # [(kw, cin), cout]
    w3 = np.ascontiguousarray(w3, dtype=f)
    # LN gains folded into projection weights; LN biases into proj biases
    qwg = (q_w * nq_g[None, :]).astype(f)
    kwg = (k_w * nkv_g[None, :]).astype(f)
    vwg = (v_w * nkv_g[None, :]).astype(f)
    qb_fold = (q_b + q_w @ nq_b).astype(f)
    kb_fold = (k_b + k_w @ nkv_b).astype(f)
    vb_fold = (v_b + v_w @ nkv_b).astype(f)
    gwg = (gate_w[0] * nq_g).astype(f)
    gb = float(gate_w[0] @ nq_b)
    cvecs = np.stack([
        tm_b1, tm_ln1_g, tm_ln1_b, tm_b2, -tm_ln2_g, -tm_ln2_b, conv_b, gwg,
    ], axis=1).astype(f)                          # [256, 8]
    dvecs = np.stack([qb_fold, kb_fold], axis=1).astype(f)
    ob_eff = (o_b + vb_fold @ o_w.T).astype(f)
    ow2 = np.zeros((33, NH, C), f)
    for h in range(NH):
        ow2[0:32, h, :] = o_w[:, 32 * h:32 * h + 32].T
        ow2[32, h, :] = ob_eff / NH
    nogr = np.ascontiguousarray(no_g[None, :], dtype=f)
    gbneg = np.full((MC, 1), -gb, f)

    per_core = []
    for core in range(8):
        b, k = core // 4, core % 4
        xb = np.ascontiguousarray(x[b].reshape(C, S), dtype=f)
        xqc = np.ascontiguousarray(xb[:, NQ * k:NQ * (k + 1)])
        sel = np.zeros((128, 2, W), f)
        if k == 0:
            sel[:, 0, :] = 1.0
        if k == 3:
            sel[:, 1, :] = 1.0
        per_core.append({
            "xk": xb,
            "xq": xqc,
            "xqres2": np.ascontiguousarray(xqc.T + no_b[None, :]),
            "text": np.ascontiguousarray(text_feature[b][:, None], dtype=f),
            "tmw1": np.ascontiguousarray(tm_w1.T, dtype=f),
            "tmw2": np.ascontiguousarray(tm_w2.T, dtype=f),
            "cvecs": cvecs, "dvecs": dvecs, "pe": pe, "w3": w3,
            "qwT": np.ascontiguousarray(qwg.T),
            "kwT": np.ascontiguousarray(kwg.T),
            "vwT": np.ascontiguousarray(vwg.T),
            "ow2": ow2, "nogr": nogr, "selmask": sel, "gbneg": gbneg,
        })
    return per_core


_NC_CACHE = {}


def get_nc():
    if "nc" not in _NC_CACHE:
        _NC_CACHE["nc"] = build_bass()
    return _NC_CACHE["nc"]


def kernel(**inputs):
    inputs = {k: np.asarray(v, dtype=np.float32) for k, v in inputs.items()}
    in_maps = _host_inputs(**inputs)
    nc = get_nc()
    res = run_bass_kernel_spmd(nc, in_maps, core_ids=list(range(8)))
    x = inputs["x"]
    out = np.empty((B, C, H, W), np.float32)
    for b in range(B):
        blocks = [res.results[4 * b + k]["y"] for k in range(4)]  # [NQ, C] each
        tok = np.concatenate(blocks, axis=0)                      # [S, C]
        out[b] = tok.T.reshape(C, H, W)
    return out


# revision 8
# speedup vs baseline: 1.2532x; 1.0835x over previous
"""Trainium2 Bass kernel for nn_EnhancedTextAttentionBlock.

Self-contained: takes FULL inputs (as in reference.setup_inputs()), shards
across 8 NeuronCores internally, returns the FULL [2, 256, 48, 48] output.

Sharding: core c handles batch b = c // 4 and query-token block k = c % 4
(576 of the 2304 spatial tokens). K/V are computed for the full token set on
every core; a single SPMD program serves all 8 cores with no collectives.

Key structure (all exact algebraic restructurings, except the fp32->f32r
matmul dtype and a Newton-refined inverse-sqrt, both far inside the error
budget):
- pe depends only on (c, w): the 3x3 conv collapses to 3 distinct rows
  (top/mid/bottom) computed as small matmuls.
- LN gains/biases of nq/nkv are folded into the q/k/v projection weights on
  the host; the kernel only applies the (x - mu) * rsqrt(var) part.
- rsqrt everywhere via DVE integer fast-inverse-sqrt + 3 Newton steps: the
  kernel then needs only {Exp, Identity, Relu, Square} activations, which
  live in ONE activation table (no table ping-pong).
- LN statistics are computed with ones-matmuls, then repacked [1,S] ->
  [128,S/128] by DMA so the scalar math runs 128-wide.
- Scores read the c-major K/Q projections directly as 32-partition slices
  (PE tile_position); heads at partition offset 96 are staged through a
  small SBUF->SBUF DMA (offsets are limited to 0/32/64) and scheduled last.
- v carries a ones-column so softmax denominators l ride through the AV
  matmul; av is normalized by 1/l BEFORE the out-projection, so the 8 heads
  accumulate in PSUM and o_b (+ v_b @ o_w.T) folds into a 33rd row of the
  out-projection weights.
- Softmax max-subtraction is skipped: LN'd activations through 0.02-scale
  weights keep |scores| small enough for exact fp32 exp.
"""
import math
import numpy as np

import concourse.bass as bass
import concourse.tile as tile
from concourse import bacc, mybir
from concourse.bass_utils import run_bass_kernel_spmd

F32 = mybir.dt.float32
F32R = mybir.dt.float32r
I32 = mybir.dt.int32
AF = mybir.ActivationFunctionType
OP = mybir.AluOpType

B, C, H, W, T = 2, 256, 48, 48, 512
NH, HD = 8, 32
S = H * W              # 2304 tokens
NQ = S // 4            # 576 q tokens per core
SCALE = HD ** -0.5
IT = 288               # q block (two per core)
MC = 96                # epilogue chunk
EPS = 1e-5
MAGIC = 0x5F3759DF

# cvecs column indices (c-major [256, 1] vectors packed into one input)
CV_TMB1, CV_L1G, CV_L1B, CV_TMB2, CV_L2GN, CV_L2BN, CV_CONVB, CV_GWG = range(8)

# head h -> 32-channel slice of the c-major projections: chunk dc = h // 4,
# partition offset 32*(h%4).  Offset-96 heads (3, 7) are staged to kst3/qst3.
PAIRS = [(0, 1), (2, 4), (5, 6), (3, 7)]


def _fisr(nc, pool, x_ap, pshape, tag, iters=3):
    """rsqrt(x) on DVE: int bit-trick seed + Newton. x_ap: SBUF f32 AP > 0.
    Returns an F32 AP of a fresh tile."""
    P, Fn = pshape
    sh = pool.tile([P, Fn], I32, tag=f"{tag}_i")
    nc.vector.tensor_scalar(sh[:], x_ap.bitcast(I32), 1, None,
                            OP.logical_shift_right)
    nc.vector.tensor_scalar(sh[:], sh[:], -1, None, OP.bitwise_xor)
    nc.vector.tensor_scalar(sh[:], sh[:], MAGIC + 1, None, OP.add)
    y = sh.bitcast(F32)
    t = pool.tile([P, Fn], F32, tag=f"{tag}_t")
    for _ in range(iters):
        nc.vector.tensor_mul(t[:], y[:], y[:])
        nc.vector.tensor_tensor(t[:], t[:], x_ap, OP.mult)
        nc.vector.tensor_scalar(t[:], t[:], -0.5, 1.5, OP.mult, OP.add)
        nc.vector.tensor_mul(y[:], y[:], t[:])
    return y


def build_bass():
    nc = bacc.Bacc("TRN2", target_bir_lowering=False, debug=False,
                   enable_asserts=True, num_devices=8)
    di = {}

    def inp(name, shape, dt=F32):
        di[name] = nc.dram_tensor(name, shape, dt, kind="ExternalInput")
        return di[name]

    inp("xk", [C, S])
    inp("xq", [C, NQ])
    inp("xqres2", [NQ, C])          # xq residual (token-major) + no_b
    inp("text", [T, 1])
    inp("tmw1", [T, C])
    inp("tmw2", [C, C])
    inp("cvecs", [C, 8])
    inp("dvecs", [C, 2])            # qb_fold, kb_fold
    inp("pe", [C, W])
    inp("w3", [3, 768, C])
    inp("qwT", [C, C], F32R)        # (q_w * nq_g).T
    inp("kwT", [C, C], F32R)        # (k_w * nkv_g).T
    inp("vwT", [C, C], F32R)        # (v_w * nkv_g).T
    inp("ow2", [33, NH, C], F32R)   # per-head o_w rows + ob_eff/8 row
    inp("nogr", [1, C])             # no_g row
    inp("selmask", [128, 2, W])     # top/bottom q-edge masks for this core
    inp("gbneg", [MC, 1])           # -(gate_w @ nq_b), prebroadcast
    y = nc.dram_tensor("y", [NQ, C], F32, kind="ExternalOutput")

    with tile.TileContext(nc) as tc:
        _build_tile(nc, tc, di, y)
    nc.compile()
    return nc


def _build_tile(nc, tc, di, y):
    with tc.tile_pool(name="cons", bufs=1) as cons:
        # ---- persistent tiles ----
        ones_sb = cons.tile([128, 1], F32R)
        nc.vector.memset(ones_sb[:], 1.0)
        cv = cons.tile([128, 2, 8], F32)
        nc.sync.dma_start(out=cv, in_=di["cvecs"].rearrange("(c p) v -> p c v", p=128))
        dv = cons.tile([128, 2, 2], F32)
        nc.sync.dma_start(out=dv, in_=di["dvecs"].rearrange("(c p) v -> p c v", p=128))
        pe_sb = cons.tile([128, 2, W], F32)
        nc.sync.dma_start(out=pe_sb, in_=di["pe"].rearrange("(c p) w -> p c w", p=128))
        qw_sb = cons.tile([128, 2, C], F32R)
        kw_sb = cons.tile([128, 2, C], F32R)
        vw_sb = cons.tile([128, 2, C], F32R)
        ow_sb = cons.tile([33, NH, C], F32R)
        nogr_sb = cons.tile([1, C], F32)
        sel_sb = cons.tile([128, 2, W], F32)
        gb_sb = cons.tile([MC, 1], F32)
        posrow = cons.tile([128, 2, 3, W], F32)   # (cc, rowtype, w)
        dtop = cons.tile([128, 2, W], F32)
        dbot = cons.tile([128, 2, W], F32)
        kst = cons.tile([128, 2, S], F32R)
        kst3 = cons.tile([32, 2, S], F32R)        # heads 3, 7
        qst = cons.tile([128, 2, NQ], F32R)
        qst3 = cons.tile([32, 2, NQ], F32R)
        v_tok = cons.tile([128, 18, NH, 33], F32R)
        avn_all = cons.tile([33, NH, NQ], F32R)
        gate_sb = cons.tile([MC, 6], F32)
        xqres_sb = cons.tile([MC, 6, C], F32)

        # ================= prologue ==================
        with tc.tile_pool(name="ph", bufs=1) as ph, \
             tc.tile_pool(name="pps", bufs=2, space="PSUM") as pps, \
             tc.tile_pool(name="bps", bufs=1, space="PSUM") as bps, \
             tc.tile_pool(name="spps", bufs=1, space="PSUM") as spps:
            # DMAs in order of first use: MLP inputs, conv inputs, tokens
            text_sb = ph.tile([128, 4, 1], F32)
            nc.sync.dma_start(out=text_sb,
                              in_=di["text"].rearrange("(k p) o -> p k o", p=128))
            w1_sb = ph.tile([128, 4, C], F32)
            nc.sync.dma_start(out=w1_sb,
                              in_=di["tmw1"].rearrange("(k p) d -> p k d", p=128))
            w2_sb = ph.tile([128, 2, C], F32)
            nc.sync.dma_start(out=w2_sb,
                              in_=di["tmw2"].rearrange("(k p) d -> p k d", p=128))
            w3_sb = ph.tile([128, 3, 6, C], F32, tag="phC")
            nc.sync.dma_start(out=w3_sb,
                              in_=di["w3"].rearrange("t (j p) m -> p t j m", p=128))
            xq_sb = ph.tile([128, 2, NQ], F32)
            nc.sync.dma_start(out=xq_sb,
                              in_=di["xq"].rearrange("(c p) s -> p c s", p=128))
            nc.sync.dma_start(out=sel_sb, in_=di["selmask"][:, :, :])
            xk_sb = ph.tile([128, 2, S], F32, tag="phA")
            nc.sync.dma_start(out=xk_sb,
                              in_=di["xk"].rearrange("(c p) s -> p c s", p=128))
            nc.sync.dma_start(out=qw_sb, in_=di["qwT"].rearrange("(c p) d -> p c d", p=128))
            nc.sync.dma_start(out=kw_sb, in_=di["kwT"].rearrange("(c p) d -> p c d", p=128))
            nc.sync.dma_start(out=vw_sb, in_=di["vwT"].rearrange("(c p) d -> p c d", p=128))
            nc.sync.dma_start(out=ow_sb, in_=di["ow2"][:, :, :])
            nc.sync.dma_start(out=nogr_sb, in_=di["nogr"][:, :])
            nc.sync.dma_start(out=gb_sb, in_=di["gbneg"][:, :])
            nc.sync.dma_start(out=xqres_sb,
                              in_=di["xqres2"].rearrange("(k p) c -> p k c", p=MC))

            # ---- text modulation MLP (c-major) ----
            def cmajor_mlp_layer(x_col, w_sb, nkc, bias_col, tag):
                h_col = ph.tile([128, 2, 1], F32, tag=f"{tag}_h")
                for c2c in range(2):
                    h_ps = pps.tile([128, 1], F32, tag="projps")
                    for kc in range(nkc):
                        nc.tensor.matmul(
                            h_ps[:, :], w_sb[:, kc, c2c * 128:(c2c + 1) * 128],
                            x_col[:, kc, :], start=(kc == 0), stop=(kc == nkc - 1))
                    nc.scalar.activation(h_col[:, c2c, :], h_ps[:, :], AF.Identity,
                                         bias=bias_col[:, c2c, :])
                return h_col

            def cmajor_ln_rs(h_col, tag):
                # 256-dim stats of [128, 2, 1] -> broadcast [128,1] rs, murs
                sum_ps = spps.tile([1, 1], F32, tag="stsum")
                sq_ps = spps.tile([1, 1], F32, tag="stsq")
                hsq = ph.tile([128, 2, 1], F32R, tag=f"{tag}_hsq")
                nc.scalar.activation(hsq[:], h_col[:], AF.Square)
                for cc in range(2):
                    nc.tensor.matmul(sum_ps[:, :], ones_sb[:],
                                     h_col.bitcast(F32R)[:, cc, :],
                                     start=(cc == 0), stop=(cc == 1))
                    nc.tensor.matmul(sq_ps[:, :], ones_sb[:], hsq[:, cc, :],
                                     start=(cc == 0), stop=(cc == 1))
                mu1 = ph.tile([1, 2], F32, tag=f"{tag}_mu1")
                nc.vector.tensor_scalar_mul(mu1[:, 0:1], sum_ps[:, :], 1.0 / 256.0)
                nc.vector.tensor_scalar_mul(mu1[:, 1:2], sq_ps[:, :], 1.0 / 256.0)
                var1 = ph.tile([1, 1], F32, tag=f"{tag}_var1")
                nc.vector.tensor_mul(var1[:], mu1[:, 0:1], mu1[:, 0:1])
                nc.vector.tensor_tensor(var1[:], mu1[:, 1:2], var1[:], OP.subtract)
                nc.vector.tensor_scalar(var1[:], var1[:], EPS, None, OP.add)
                rs1 = _fisr(nc, ph, var1[:], (1, 1), f"{tag}_f")
                murs1 = ph.tile([1, 1], F32, tag=f"{tag}_mrs")
                nc.vector.tensor_tensor(murs1[:], mu1[:, 0:1], rs1[:], OP.mult)
                rs_b = ph.tile([128, 1], F32, tag=f"{tag}_rsb")
                nc.gpsimd.partition_broadcast(rs_b[:], rs1[:])
                murs_b = ph.tile([128, 1], F32, tag=f"{tag}_mub")
                nc.gpsimd.partition_broadcast(murs_b[:], murs1[:])
                return rs_b, murs_b

            h1 = cmajor_mlp_layer(text_sb, w1_sb, 4, cv[:, :, CV_TMB1:CV_TMB1 + 1], "l1")
            rs_b, murs_b = cmajor_ln_rs(h1, "l1")
            h1n = ph.tile([128, 2, 1], F32, tag="h1n")
            mod = ph.tile([128, 2, 1], F32, tag="mod")
            for cc in range(2):
                nc.vector.tensor_scalar(h1n[:, cc, :], h1[:, cc, :], rs_b[:],
                                        murs_b[:], OP.mult, OP.subtract)
                nc.scalar.activation(h1n[:, cc, :], h1n[:, cc, :], AF.Relu,
                                     bias=cv[:, cc, CV_L1B:CV_L1B + 1],
                                     scale=cv[:, cc, CV_L1G:CV_L1G + 1])
            h2 = cmajor_mlp_layer(h1n, w2_sb, 2, cv[:, :, CV_TMB2:CV_TMB2 + 1], "l2")
            rs2_b, murs2_b = cmajor_ln_rs(h2, "l2")
            for cc in range(2):
                nc.vector.tensor_scalar(mod[:, cc, :], h2[:, cc, :], rs2_b[:],
                                        murs2_b[:], OP.mult, OP.subtract)
                # sigmoid(z) = 1/(1 + exp(-(g*xn+b))) via pre-negated g, b
                nc.scalar.activation(mod[:, cc, :], mod[:, cc, :], AF.Exp,
                                     bias=cv[:, cc, CV_L2BN:CV_L2BN + 1],
                                     scale=cv[:, cc, CV_L2GN:CV_L2GN + 1])
                nc.vector.tensor_scalar(mod[:, cc, :], mod[:, cc, :], 1.0, None, OP.add)
                nc.vector.reciprocal(mod[:, cc, :], mod[:, cc, :])

            # ---- conditional positional rows: 3 distinct conv rows ----
            inrow = ph.tile([128, 2, W], F32)
            for cc in range(2):
                nc.vector.tensor_scalar_mul(inrow[:, cc, :], pe_sb[:, cc, :],
                                            mod[:, cc, 0:1])
            im2 = ph.tile([128, 6, W], F32)
            nc.vector.memset(im2[:], 0.0)
            for kw in range(3):
                for cc in range(2):
                    j = kw * 2 + cc
                    if kw == 0:
                        nc.vector.tensor_copy(im2[:, j, 1:W], inrow[:, cc, 0:W - 1])
                    elif kw == 1:
                        nc.vector.tensor_copy(im2[:, j, :], inrow[:, cc, :])
                    else:
                        nc.vector.tensor_copy(im2[:, j, 0:W - 1], inrow[:, cc, 1:W])
            cps = pps.tile([128, 3, 2, W], F32, tag="projps")
            for t in range(3):
                for oc in range(2):
                    for j in range(6):
                        nc.tensor.matmul(cps[:, t, oc, :],
                                         w3_sb.bitcast(F32R)[:, t, j, oc * 128:(oc + 1) * 128],
                                         im2.bitcast(F32R)[:, j, :],
                                         start=(j == 0), stop=(j == 5))
            for cc in range(2):
                nc.scalar.activation(posrow[:, cc, :, :], cps[:, :, cc, :], AF.Identity,
                                     bias=cv[:, cc, CV_CONVB:CV_CONVB + 1])
                nc.vector.tensor_sub(dtop[:, cc, :], posrow[:, cc, 0, :],
                                     posrow[:, cc, 1, :])
                nc.vector.tensor_sub(dbot[:, cc, :], posrow[:, cc, 2, :],
                                     posrow[:, cc, 1, :])

            # ---- tokens (c-major); big mid-row adds split DVE / Pool ----
            tok = ph.tile([128, 2, S], F32, tag="phB")
            for cc in range(2):
                eng = nc.vector if cc == 0 else nc.gpsimd
                eng.tensor_add(tok[:, cc, 0:W], xk_sb[:, cc, 0:W],
                               posrow[:, cc, 0, :])
                mid = posrow[:, cc, 1:2, :].to_broadcast([128, H - 2, W])
                eng.tensor_tensor(
                    tok[:, cc, W:S - W].rearrange("p (h w) -> p h w", w=W),
                    xk_sb[:, cc, W:S - W].rearrange("p (h w) -> p h w", w=W),
                    mid, OP.add)
                eng.tensor_add(tok[:, cc, S - W:S], xk_sb[:, cc, S - W:S],
                               posrow[:, cc, 2, :])
            tokq = ph.tile([128, 2, NQ], F32)
            edge = ph.tile([128, W], F32, tag="edge")
            for cc in range(2):
                mid = posrow[:, cc, 1:2, :].to_broadcast([128, NQ // W, W])
                nc.vector.tensor_tensor(
                    tokq[:, cc, :].rearrange("p (h w) -> p h w", w=W),
                    xq_sb[:, cc, :].rearrange("p (h w) -> p h w", w=W),
                    mid, OP.add)
                nc.vector.tensor_mul(edge[:], sel_sb[:, 0, :], dtop[:, cc, :])
                nc.vector.tensor_add(tokq[:, cc, 0:W], tokq[:, cc, 0:W], edge[:])
                nc.vector.tensor_mul(edge[:], sel_sb[:, 1, :], dbot[:, cc, :])
                nc.vector.tensor_add(tokq[:, cc, NQ - W:NQ], tokq[:, cc, NQ - W:NQ],
                                     edge[:])

            # ---- LN stats: ones-matmul sums, packed [128, n] scalar math ----
            def ln_stats_rows(x_t, n_free, npk, tag, bc):
                """x_t: [128, 2, n_free] f32. Writes rs/murs rows into
                bc[0:1, 0/1, :] (partition 0 of the broadcast dest).
                npk = packed columns (4 per 512-chunk)."""
                pk = ph.tile([128, 2, npk], F32, tag=f"{tag}_pk")
                nc.vector.memset(pk[:], 1.0)
                nhalf = (n_free + 511) // 512
                for hf in range(nhalf):
                    f0 = hf * 512
                    fn = min(512, n_free - f0)
                    sum_ps = spps.tile([1, 512], F32, tag="stsum")
                    sq_ps = spps.tile([1, 512], F32, tag="stsq")
                    for cc in range(2):
                        sq = ph.tile([128, 512], F32R, tag=f"sqc{cc}")
                        nc.scalar.activation(sq[:, :fn], x_t[:, cc, f0:f0 + fn],
                                             AF.Square)
                        nc.tensor.matmul(sum_ps[:, :fn], ones_sb[:],
                                         x_t.bitcast(F32R)[:, cc, f0:f0 + fn],
                                         start=(cc == 0), stop=(cc == 1))
                        nc.tensor.matmul(sq_ps[:, :fn], ones_sb[:], sq[:, :fn],
                                         start=(cc == 0), stop=(cc == 1))
                    # stage PSUM rows to SBUF (DMA cannot read PSUM), then
                    # pack t = f0 + 4p + j  ->  pk[p, 4*hf + j]
                    stage = ph.tile([1, 2, 512], F32, tag=f"stg{hf % 2}")
                    nc.scalar.activation(stage[:, 0, 0:fn], sum_ps[:, 0:fn],
                                         AF.Identity)
                    nc.gpsimd.tensor_copy(stage[:, 1, 0:fn], sq_ps[:, 0:fn])
                    np_rows = fn // 4
                    nc.sync.dma_start(
                        out=pk[0:np_rows, 0, 4 * hf:4 * hf + 4],
                        in_=stage[0:1, 0, 0:fn].rearrange("o (p j) -> o p j", j=4))
                    nc.sync.dma_start(
                        out=pk[0:np_rows, 1, 4 * hf:4 * hf + 4],
                        in_=stage[0:1, 1, 0:fn].rearrange("o (p j) -> o p j", j=4))
                # packed math: mu = s/256, ex2 = sq/256, var = ex2 - mu^2
                m = ph.tile([128, 2, npk], F32, tag=f"{tag}_m")
                nc.vector.tensor_scalar_mul(m[:], pk[:], 1.0 / 256.0)
                varx = ph.tile([128, npk], F32, tag=f"{tag}_v")
                nc.vector.tensor_mul(varx[:], m[:, 0, :], m[:, 0, :])
                nc.vector.tensor_tensor(varx[:], m[:, 1, :], varx[:], OP.subtract)
                nc.vector.tensor_scalar(varx[:], varx[:], EPS, None, OP.add)
                rs_pk = _fisr(nc, ph, varx[:], (128, npk), f"{tag}_f")
                murs_pk = ph.tile([128, npk], F32, tag=f"{tag}_ms")
                nc.vector.tensor_tensor(murs_pk[:], m[:, 0, :], rs_pk[:], OP.mult)
                for hf in range(nhalf):
                    f0 = hf * 512
                    fn = min(512, n_free - f0)
                    np_rows = fn // 4
                    nc.sync.dma_start(
                        out=bc[0:1, 0, f0:f0 + fn].rearrange("o (p j) -> o p j", j=4),
                        in_=rs_pk[0:np_rows, 4 * hf:4 * hf + 4])
                    nc.sync.dma_start(
                        out=bc[0:1, 1, f0:f0 + fn].rearrange("o (p j) -> o p j", j=4),
                        in_=murs_pk[0:np_rows, 4 * hf:4 * hf + 4])

            # rs/murs rows land on partition 0 of bc tiles; they are
            # broadcast per 512-half with a rank-1 PE matmul (ones column
            # times row) into PSUM, consumed directly by the LN-core ops.
            bc_k = ph.tile([128, 2, S], F32, tag="phC")          # reuses w3 slot
            bc_q = ph.tile([128, 2, NQ], F32, tag="bcq")
            ln_stats_rows(tok, S, 20, "sk", bc_k)
            ln_stats_rows(tokq, NQ, 8, "sq", bc_q)
            ones_row = ph.tile([1, 128], F32R, tag="onesrow")
            nc.vector.memset(ones_row[:], 1.0)

            # ---- q side first: qn -> Q-proj -> qst3 + gate ----
            qn = ph.tile([128, 2, NQ], F32R, tag="xq_sb")  # xq dead after tokq
            qnf = qn.bitcast(F32)
            for (s0, stn) in ((0, 512), (512, 64)):
                bq_ps = bps.tile([128, 2, 512], F32, tag="bcps")
                for rx in range(2):
                    nc.tensor.matmul(bq_ps[:, rx, 0:stn], ones_row[:],
                                     bc_q.bitcast(F32R)[0:1, rx, s0:s0 + stn],
                                     start=True, stop=True)
                for cc in range(2):
                    nc.vector.tensor_tensor(qnf[:, cc, s0:s0 + stn],
                                            tokq[:, cc, s0:s0 + stn],
                                            bq_ps[:, 0, 0:stn], OP.mult)
                    nc.vector.tensor_tensor(qnf[:, cc, s0:s0 + stn],
                                            qnf[:, cc, s0:s0 + stn],
                                            bq_ps[:, 1, 0:stn], OP.subtract)
                for dc in range(2):
                    qp = pps.tile([128, 512], F32, tag="projps")
                    for cc in range(2):
                        nc.tensor.matmul(qp[:, :stn],
                                         qw_sb[:, cc, dc * 128:(dc + 1) * 128],
                                         qn[:, cc, s0:s0 + stn],
                                         start=(cc == 0), stop=(cc == 1))
                    nc.scalar.activation(qst[:, dc, s0:s0 + stn], qp[:, :stn],
                                         AF.Identity, bias=dv[:, dc, 0:1])
            nc.sync.dma_start(out=qst3[:, :, :], in_=qst[96:128, :, :])

            # ---- k side per 512-half: kn -> K-proj -> V-proj, pipelined ----
            kn = ph.tile([128, 2, S], F32R, tag="phA")           # reuses xk slot
            knf = kn.bitcast(F32)
            nc.vector.tensor_copy(
                v_tok[:, :, :, 32:33],
                ones_sb[:, None, None, :].to_broadcast([128, 18, NH, 1]))
            STILE = [512, 512, 512, 512, 256]
            for hf, stn in enumerate(STILE):
                s0 = 512 * hf
                bk_ps = bps.tile([128, 2, 512], F32, tag="bcps")
                for rx in range(2):
                    nc.tensor.matmul(bk_ps[:, rx, 0:stn], ones_row[:],
                                     bc_k.bitcast(F32R)[0:1, rx, s0:s0 + stn],
                                     start=True, stop=True)
                for cc in range(2):
                    eng = nc.vector if cc == 0 else nc.gpsimd
                    eng.tensor_tensor(knf[:, cc, s0:s0 + stn],
                                      tok[:, cc, s0:s0 + stn],
                                      bk_ps[:, 0, 0:stn], OP.mult)
                    eng.tensor_tensor(knf[:, cc, s0:s0 + stn],
                                      knf[:, cc, s0:s0 + stn],
                                      bk_ps[:, 1, 0:stn], OP.subtract)
                for dc in range(2):
                    kp = pps.tile([128, 512], F32, tag="projps")
                    for cc in range(2):
                        nc.tensor.matmul(kp[:, :stn],
                                         kw_sb[:, cc, dc * 128:(dc + 1) * 128],
                                         kn[:, cc, s0:s0 + stn],
                                         start=(cc == 0), stop=(cc == 1))
                    nc.scalar.activation(kst[:, dc, s0:s0 + stn], kp[:, :stn],
                                         AF.Identity, bias=dv[:, dc, 1:2])
                for sc in range(s0 // 128, (s0 + stn) // 128):
                    vp = pps.tile([128, 512], F32, tag="projps")
                    for cc in range(2):
                        nc.tensor.matmul(vp[:, 0:C],
                                         kn[:, cc, sc * 128:(sc + 1) * 128],
                                         vw_sb[:, cc, :], start=(cc == 0), stop=(cc == 1))
                    eng = nc.vector if sc % 2 == 0 else nc.gpsimd
                    eng.tensor_copy(
                        v_tok[:, sc, :, 0:32],
                        vp[:, 0:C].rearrange("p (h d) -> p h d", d=32))
            # stage offset-96 heads (3, 7) to partition-0 tiles
            nc.sync.dma_start(out=kst3[:, :, :], in_=kst[96:128, :, :])
            # gate logits -> exp(-(z + gb))
            eg_sb = ph.tile([MC, 6], F32, tag="eg")
            for ic in range(6):
                gp = pps.tile([MC, 1], F32, tag="projps")
                for cc in range(2):
                    nc.tensor.matmul(gp[:, :],
                                     qn[:, cc, ic * MC:(ic + 1) * MC],
                                     cv.bitcast(F32R)[:, cc, CV_GWG:CV_GWG + 1],
                                     start=(cc == 0), stop=(cc == 1))
                nc.scalar.activation(eg_sb[:, ic:ic + 1], gp[:, :], AF.Exp,
                                     scale=-1.0, bias=gb_sb[:, :])
            nc.vector.tensor_scalar(gate_sb[:], eg_sb[:], 1.0, None, OP.add)
            nc.vector.reciprocal(gate_sb[:], gate_sb[:])

        # ================= attention ==================
        def kslc(h, jc):
            if h == 3 or h == 7:
                return kst3[:, h // 4, jc * 128:(jc + 1) * 128]
            return kst[32 * (h % 4):32 * (h % 4) + 32, h // 4,
                       jc * 128:(jc + 1) * 128]

        def qslc(h, it):
            if h == 3 or h == 7:
                return qst3[:, h // 4, it * IT:(it + 1) * IT]
            return qst[32 * (h % 4):32 * (h % 4) + 32, h // 4,
                       it * IT:(it + 1) * IT]

        with tc.tile_pool(name="atte", bufs=4) as atte, \
             tc.tile_pool(name="ps_s", bufs=2, space="PSUM") as ps_s, \
             tc.tile_pool(name="ps_av", bufs=2, space="PSUM") as ps_av:
            for (hA, hB) in PAIRS:
                for it in range(2):
                    av_ps = ps_av.tile([33, 2, 512], F32, tag="avps")
                    pend = []

                    def emit_av(e_jc):
                        e_sb, jc = e_jc
                        for hh, h in enumerate((hA, hB)):
                            nc.tensor.matmul(
                                av_ps[:, hh, 0:IT], v_tok[:, jc, h, :],
                                e_sb[:, hh, :], start=(jc == 0), stop=(jc == 17))

                    for jc in range(18):
                        s_ps = ps_s.tile([128, 2, 512], F32, tag="sps")
                        for hh, h in enumerate((hA, hB)):
                            nc.tensor.matmul(s_ps[:, hh, 0:IT], kslc(h, jc),
                                             qslc(h, it), start=True, stop=True)
                        e_sb = atte.tile([128, 2, IT], F32R, tag="esb")
                        nc.scalar.activation(e_sb[:, :, :], s_ps[:, :, 0:IT],
                                             AF.Exp, scale=SCALE)
                        pend.append((e_sb, jc))
                        if len(pend) > 2:
                            emit_av(pend.pop(0))
                    for e_jc in pend:
                        emit_av(e_jc)
                    # normalize by 1/l (row 32 of av_ps) into avn_all
                    r1 = atte.tile([1, 2, IT], F32, tag="rsb")
                    nc.vector.reciprocal(r1[:], av_ps[32:33, :, 0:IT])
                    rb = atte.tile([33, 2, IT], F32, tag="rbb")
                    nc.gpsimd.partition_broadcast(rb[:, 0, :], r1[0:1, 0, :])
                    nc.gpsimd.partition_broadcast(rb[:, 1, :], r1[0:1, 1, :])
                    for hh, h in enumerate((hA, hB)):
                        nc.vector.tensor_tensor(
                            avn_all.bitcast(F32)[:, h, it * IT:(it + 1) * IT],
                            av_ps[:, hh, 0:IT], rb[:, hh, :], OP.mult)

        # ================= out-projection + epilogue ==================
        with tc.tile_pool(name="ep", bufs=2) as ep, \
             tc.tile_pool(name="epc", bufs=1) as epc, \
             tc.tile_pool(name="ps_o", bufs=3, space="PSUM") as ps_o:
            og_all = epc.tile([MC, 6, C], F32)
            mv_all = epc.tile([MC, 6, 2], F32)
            for ch in range(6):
                o_ps = ps_o.tile([MC, C], F32, tag="ops")
                for h in range(NH):
                    nc.tensor.matmul(o_ps[:, :],
                                     avn_all[:, h, ch * MC:(ch + 1) * MC],
                                     ow_sb[:, h, :], start=(h == 0), stop=(h == 7))
                nc.scalar.activation(og_all[:, ch, :], o_ps[:, :], AF.Identity,
                                     scale=gate_sb[:, ch:ch + 1])
                stats = ep.tile([MC, nc.vector.BN_STATS_DIM], F32, tag="bst")
                nc.vector.bn_stats(stats[:], og_all[:, ch, :])
                nc.vector.bn_aggr(mv_all[:, ch, :], stats[:])
            varx = epc.tile([MC, 6], F32)
            nc.vector.tensor_scalar(varx[:], mv_all[:, :, 1], EPS, None, OP.add)
            rs_all = _fisr(nc, epc, varx[:], (MC, 6), "ef")
            murs_all = epc.tile([MC, 6], F32)
            nc.vector.tensor_tensor(murs_all[:], mv_all[:, :, 0], rs_all[:], OP.mult)
            nogb = epc.tile([MC, C], F32)
            nc.gpsimd.partition_broadcast(nogb[:], nogr_sb[0:1, :])
            for ch in range(6):
                t2 = ep.tile([MC, C], F32, tag="ept2")
                nc.vector.tensor_scalar(t2[:], og_all[:, ch, :],
                                        rs_all[:, ch:ch + 1],
                                        murs_all[:, ch:ch + 1],
                                        OP.mult, OP.subtract)
                nc.vector.tensor_mul(t2[:], t2[:], nogb[:])
                nc.vector.tensor_add(t2[:], t2[:], xqres_sb[:, ch, :])
                nc.sync.dma_start(
                    out=y.rearrange("(k p) c -> p k c", p=MC)[:, ch, :], in_=t2[:])


def _host_inputs(x, text_feature, tm_w1, tm_b1, tm_ln1_g, tm_ln1_b, tm_w2, tm_b2,
                 tm_ln2_g, tm_ln2_b, conv_w, conv_b, q_w, q_b, k_w, k_b, v_w, v_b,
                 o_w, o_b, gate_w, nq_g, nq_b, nkv_g, nkv_b, no_g, no_b):
    f = np.float32
    # pe table (depends only on (c, w); faithful to reference)
    div = np.exp(np.arange(C // 2, dtype=f) * (-math.log(10000.0) / (C // 2)))
    wpos = np.arange(W, dtype=f)
    s = np.sin(wpos[None, :] * div[:, None])
    c = np.cos(wpos[None, :] * div[:, None])
    pe = np.stack([s, c], axis=1).reshape(C, W).astype(f)
    # kh-collapsed conv kernels: top(kh 1,2), mid(all), bot(kh 0,1)
    w3 = np.stack([
        conv_w[:, :, 1, :] + conv_w[:, :, 2, :],
        conv_w.sum(axis=2),
        conv_w[:, :, 0, :] + conv_w[:, :, 1, :],
    ]).astype(f)                                  # [3, Cout, Cin, kw]
    w3 = w3.transpose(0, 3, 2, 1).reshape(3, 768, C)  # [(kw, cin), cout]
    w3 = np.ascontiguousarray(w3, dtype=f)
    # LN gains folded into projection weights; LN biases into proj biases
    qwg = (q_w * nq_g[None, :]).astype(f)
    kwg = (k_w * nkv_g[None, :]).astype(f)
    vwg = (v_w * nkv_g[None, :]).astype(f)
    qb_fold = (q_b + q_w @ nq_b).astype(f)
    kb_fold = (k_b + k_w @ nkv_b).astype(f)
    vb_fold = (v_b + v_w @ nkv_b).astype(f)
    gwg = (gate_w[0] * nq_g).astype(f)
    gb = float(gate_w[0] @ nq_b)
    cvecs = np.stack([
        tm_b1, tm_ln1_g, tm_ln1_b, tm_b2, -tm_ln2_g, -tm_ln2_b, conv_b, gwg,
    ], axis=1).astype(f)                          # [256, 8]
    dvecs = np.stack([qb_fold, kb_fold], axis=1).astype(f)
    ob_eff = (o_b + vb_fold @ o_w.T).astype(f)
    ow2 = np.zeros((33, NH, C), f)
    for h in range(NH):
        ow2[0:32, h, :] = o_w[:, 32 * h:32 * h + 32].T
        ow2[32, h, :] = ob_eff / NH
    nogr = np.ascontiguousarray(no_g[None, :], dtype=f)
    gbneg = np.full((MC, 1), -gb, f)

    per_core = []
    for core in range(8):
        b, k = core // 4, core % 4
        xb = np.ascontiguousarray(x[b].reshape(C, S), dtype=f)
        xqc = np.ascontiguousarray(xb[:, NQ * k:NQ * (k + 1)])
        sel = np.zeros((128, 2, W), f)
        if k == 0:
            sel[:, 0, :] = 1.0
        if k == 3:
            sel[:, 1, :] = 1.0
        per_core.append({
            "xk": xb,
            "xq": xqc,
            "xqres2": np.ascontiguousarray(xqc.T + no_b[None, :]),
            "text": np.ascontiguousarray(text_feature[b][:, None], dtype=f),
            "tmw1": np.ascontiguousarray(tm_w1.T, dtype=f),
            "tmw2": np.ascontiguousarray(tm_w2.T, dtype=f),
            "cvecs": cvecs, "dvecs": dvecs, "pe": pe, "w3": w3,
            "qwT": np.ascontiguousarray(qwg.T),
            "kwT": np.ascontiguousarray(kwg.T),
            "vwT": np.ascontiguousarray(vwg.T),
            "ow2": ow2, "nogr": nogr, "selmask": sel, "gbneg": gbneg,
        })
    return per_core


_NC_CACHE = {}


def get_nc():
    if "nc" not in _NC_CACHE:
        _NC_CACHE["nc"] = build_bass()
    return _NC_CACHE["nc"]


def kernel(**inputs):
    inputs = {k: np.asarray(v, dtype=np.float32) for k, v in inputs.items()}
    in_maps = _host_inputs(**inputs)
    nc = get_nc()
    res = run_bass_kernel_spmd(nc, in_maps, core_ids=list(range(8)))
    x = inputs["x"]
    out = np.empty((B, C, H, W), np.float32)
    for b in range(B):
        blocks = [res.results[4 * b + k]["y"] for k in range(4)]  # [NQ, C] each
        tok = np.concatenate(blocks, axis=0)                      # [S, C]
        out[b] = tok.T.reshape(C, H, W)
    return out
